# revision 30
# baseline (speedup 1.0000x reference)
"""Batch-hard triplet loss on 8 Trainium2 NeuronCores (Bass/Tile).

Strategy (data-parallel over anchor rows):
  Each core owns R = B/8 anchor rows and mines hard pos/neg from the score
  block  S[m, n] = ||e_m - e_n||^2 + C * [label_m == label_n]  without ever
  materializing indices or gathers:

      hard-positive d2 = rowmax(S) - C - sq_m   (same-label entries at d2+C)
      hard-negative d2 = rowmin(S)     - sq_m   (diff-label entries at d2)

  With C (32768) larger than any squared distance, the diagonal sits at
  exactly ~C: never the max when a real positive exists, never the min when
  a real negative exists; degenerate rows are masked by the host-computed
  `valid`.  Row max/min are tie-immune (values, not argmax indices).

  Default pipeline (VERSION=4):
  - PE streams the pure Gram:  q = ((-2/C)emb.T)^T @ ((-2/C)emb.T) in
    float32r (full PE rate at N=512, near-fp32 precision; the 1/C and the
    later -C/2 rescale are exact power-of-2 scalings).
  - Per-core column ROTATION (host-side roll) puts the core's own 512-col
    block at n-tile 0, so that one load doubles as the matmul stationary
    operand — no separate lhsT input or transfer.
  - The label-equality mask and +sq_n/C ride a single fused DVE op
    ((lab_bc == lab_m) + sq_bc), added to the PSUM block with the -C/2
    rescale in a second fused op, then DVE row max/min reduce.
  - lab/sq rows reach all 128 partitions via stride-0 broadcast DMA
    (GpSimd SWDGE).
  - Loss tail (sqrt via ACT, margin/relu/valid-mask via DVE, partition-sum
    via a ones-matmul) stays on device; the host only sums 4 partial sums
    per core and divides by the valid count.

  Older fallbacks kept for reference: VERSION=1 (bf16 one-hot + hi/lo sq
  k-tiles appended to the matmul), 2 (one-hot folded into the f32r stream),
  3 (v4 without rotation).  FEAT_DT=bf16 halves DMA bytes at ~bf16 matmul
  precision; default stays f32r.
"""

import numpy as np
import ml_dtypes

B = 4096
D = 2048
NCORES = 8
L = 128          # number of label classes (labels are in [0, 128))
P = 128          # partitions
NT = 512         # n-tile (matmul free dim = one PSUM bank of f32)
CBIG = 32768.0   # separation constant; must exceed max squared distance
MARGIN = 0.3

import os as _os

TRACE = False           # test.py sets this to profile
LAST_RESULT = None      # BassKernelResults of the most recent run
# "f32r" (near-fp32 matmul) or "bf16" (half the DMA)
FEAT_DT = _os.environ.get("KERNEL_FEAT_DT", "f32r")
# 1: separate bf16 onehot/sq matmul k-tiles (18 total)
# 2: onehot folded into the feature stream (17 tiles), sq added on DVE
# 3: pure 16-tile Gram on PE; same-mask (is_equal) + sq both on DVE
# 4: v3 + column rotation (own block doubles as lhsT) + on-PE broadcasts
# 5: fp8e4m3 DoubleRow matmuls (2 k-slabs/instr, 2x PE rate); +-128 one-hot
#    mask pair on PE; DVE = 2 fused (ps - sq_n/2) -> min/max passes
# 6: v5 + pre-tiled contiguous DMA (1 descriptor/tile) + ACT PSUM->SBUF copy
#    so DVE reduces run from SBUF in 2x mode
# 7: v6 + weight-stationary loop order (m, j outer; n inner over 4 PSUM
#    banks) so each DoubleRow weight load serves 4 moving streams
VERSION = int(_os.environ.get("KERNEL_VERSION", "7"))
# engine-isolation for bench ablation: "", "nope", "nodma", "nodve"
ABLATE = _os.environ.get("KERNEL_ABLATE", "")

_cache = {}


def _build(b, d, n_cores, l=L, nt=NT, repeat=1, feat=None, version=None):
    """Build + compile the per-core Bass kernel (same NEFF for all cores).

    repeat>1 emits the whole body N times (bench builds: slope timing)."""
    import concourse.mybir as mybir
    import concourse.tile as tile
    from concourse import bacc

    r = b // n_cores      # local anchor rows per core
    mc = r // P           # m-chunks of 128 anchors
    kt = d // P           # feature k-tiles
    ntil = b // nt        # n-tiles over all B columns

    if feat is None:
        feat = FEAT_DT
    if version is None:
        version = VERSION
    f32 = mybir.dt.float32
    bf16 = mybir.dt.bfloat16
    fdt = mybir.dt.float32r if feat == "f32r" else bf16

    nc = bacc.Bacc(
        "TRN2", target_bir_lowering=False, debug=False, num_devices=n_cores
    )

    if version in (5, 6, 7):
        fp8 = mybir.dt.float8e4
        kt8 = d // P + 2  # 16 emb slabs + sq-levels slab + one-hot slab
        if version >= 6:
            # pre-tiled: row (n*P + p), col (k*nt + c)
            embT2 = nc.dram_tensor(
                "embT8", [(b // nt) * P, kt8 * nt], fp8, kind="ExternalInput"
            ).ap()
        else:
            embT2 = nc.dram_tensor(
                "embT8", [kt8 * P, b], fp8, kind="ExternalInput"
            ).ap()
        ohstd = nc.dram_tensor(
            "ohstd", [P, 2 * (b // n_cores)], fp8, kind="ExternalInput"
        ).ap()
    elif version == 4:
        embT2 = nc.dram_tensor("embT2", [d, b], fdt, kind="ExternalInput").ap()
        sqfd = nc.dram_tensor("sqfd", [1, b], f32, kind="ExternalInput").ap()
        labfd = nc.dram_tensor("labfd", [1, b], f32, kind="ExternalInput").ap()
        labld = nc.dram_tensor("labld", [P, b // n_cores // P], f32,
                               kind="ExternalInput").ap()
    elif version == 3:
        embT2 = nc.dram_tensor("embT2", [d, b], fdt, kind="ExternalInput").ap()
        lhsTd = nc.dram_tensor("lhsTd", [d, r], fdt, kind="ExternalInput").ap()
        sqfd = nc.dram_tensor("sqfd", [1, b], f32, kind="ExternalInput").ap()
        labfd = nc.dram_tensor("labfd", [1, b], f32, kind="ExternalInput").ap()
        labld = nc.dram_tensor("labld", [P, b // n_cores // P], f32,
                               kind="ExternalInput").ap()
    elif version == 2:
        d2 = d + l
        embT2 = nc.dram_tensor("embT2", [d2, b], fdt, kind="ExternalInput").ap()
        lhsTd = nc.dram_tensor("lhsTd", [d2, r], fdt, kind="ExternalInput").ap()
        sqfd = nc.dram_tensor("sqfd", [1, b], f32, kind="ExternalInput").ap()
    else:
        embT2 = nc.dram_tensor("embT2", [d, b], fdt, kind="ExternalInput").ap()
        lhsTd = nc.dram_tensor("lhsTd", [d, r], fdt, kind="ExternalInput").ap()
        ohTd = nc.dram_tensor("ohTd", [l, b], bf16, kind="ExternalInput").ap()
        ohTCd = nc.dram_tensor(
            "ohTCd", [l, r], bf16, kind="ExternalInput"
        ).ap()
        sqrd = nc.dram_tensor("sqrd", [2, b], bf16, kind="ExternalInput").ap()
    sqlCd = nc.dram_tensor("sqlCd", [P, mc], f32, kind="ExternalInput").ap()
    sqld = nc.dram_tensor("sqld", [P, mc], f32, kind="ExternalInput").ap()
    vldd = nc.dram_tensor("vldd", [P, mc], f32, kind="ExternalInput").ap()
    outd = nc.dram_tensor("out", [mc, 1], f32, kind="ExternalOutput").ap()

    with tile.TileContext(nc) as tc:
        for _rep in range(repeat):
            if version == 7:
                _emit_body_v7(
                    nc, tc, embT2, ohstd, sqlCd, sqld, vldd, outd,
                    b, r, mc, d // P, ntil, nt,
                )
            elif version in (5, 6):
                _emit_body_v5(
                    nc, tc, embT2, ohstd, sqlCd, sqld, vldd, outd,
                    b, r, mc, d // P, ntil, nt, version,
                )
            elif version == 4:
                _emit_body_v4(
                    nc, tc, embT2, sqfd, labfd, labld, sqlCd, sqld,
                    vldd, outd, b, r, mc, kt, ntil, nt, fdt,
                )
            elif version == 3:
                _emit_body_v3(
                    nc, tc, embT2, lhsTd, sqfd, labfd, labld, sqlCd, sqld,
                    vldd, outd, b, r, mc, kt, ntil, nt, fdt,
                )
            elif version == 2:
                _emit_body_v2(
                    nc, tc, embT2, lhsTd, sqfd, sqlCd, sqld, vldd, outd,
                    b, r, mc, (d + l) // P, ntil, nt, fdt,
                )
            else:
                _emit_body(
                    nc, tc, embT2, lhsTd, ohTd, ohTCd, sqrd, sqlCd, sqld,
                    vldd, outd, b, r, mc, kt, ntil, nt, l, fdt,
                )

    nc.compile()
    return nc


def _emit_body_v7(
    nc, tc, embT8, ohstd, sqlCd, sqld, vldd, outd,
    b, r, mc, kt, ntil, nt,
):
    """v7: weight-stationary ordering.  All 8 rhs n-tiles stay resident in
    SBUF; for each (m-chunk, half-of-4-n-tiles), the j loop loads each
    DoubleRow weight set once and streams it against 4 moving n-tiles into 4
    PSUM banks (interleaved accumulation groups), cutting LD_WEIGHTS traffic
    8x.  Reduce chain and math identical to v6."""
    from contextlib import ExitStack

    import concourse.mybir as mybir

    f32 = mybir.dt.float32
    fp8 = mybir.dt.float8e4
    AT = mybir.AxisListType
    OP = mybir.AluOpType
    PM = mybir.MatmulPerfMode
    assert r == nt, "column rotation requires r == nt"
    kt8 = kt + 2
    npair = kt // 2
    nhalf = 4  # n-tiles per PSUM wave

    with ExitStack() as ctx:
        singles = ctx.enter_context(tc.tile_pool(name="singles", bufs=1))
        psum = ctx.enter_context(tc.tile_pool(name="psum", bufs=7, space="PSUM"))
        psum1 = ctx.enter_context(
            tc.tile_pool(name="psum1", bufs=1, space="PSUM")
        )
        small = ctx.enter_context(tc.tile_pool(name="small", bufs=2))
        rhspool = ctx.enter_context(tc.tile_pool(name="rhspool", bufs=9))
        scrpool = ctx.enter_context(tc.tile_pool(name="scrpool", bufs=4))

        embr = embT8.rearrange("(n p) (k c) -> n p k c", p=P, k=kt8)
        rhs_t = []
        for n in range(ntil):
            t = rhspool.tile([P, kt8, nt], fp8, tag="rhs", name=f"rhs{n}")
            nc.sync.dma_start(out=t, in_=embr[n])
            rhs_t.append(t)
        lhs_sb = rhs_t[0]  # rotation: own block doubles as stationary
        ohst_sb = singles.tile([P, 2, nt], fp8)
        nc.sync.dma_start(
            out=ohst_sb, in_=ohstd.rearrange("p (two m) -> p two m", two=2)
        )
        sql_sb = singles.tile([P, mc], f32)
        nc.sync.dma_start(out=sql_sb, in_=sqld)
        sqlC_sb = singles.tile([P, mc], f32)
        nc.sync.dma_start(out=sqlC_sb, in_=sqlCd)
        vld_sb = singles.tile([P, mc], f32)
        nc.sync.dma_start(out=vld_sb, in_=vldd)
        onesc = singles.tile([P, 1], f32)
        nc.vector.memset(onesc, 1.0)

        u_t = singles.tile([P, mc, ntil], f32)  # rowmin partials
        v_t = singles.tile([P, mc, ntil], f32)  # rowmax partials

        for m in range(mc):
            for h in range(ntil // nhalf):
                pss = []
                for x in range(nhalf):
                    pss.append(psum.tile([P, nt], f32, tag="ps", name="ps"))
                for j in range(npair):
                    lhsT = lhs_sb[:, 2 * j : 2 * j + 2, m * P : (m + 1) * P]
                    for x in range(nhalf):
                        nc.tensor.matmul(
                            pss[x],
                            lhsT=lhsT,
                            rhs=rhs_t[h * nhalf + x][:, 2 * j : 2 * j + 2, :],
                            start=(j == 0),
                            stop=False,
                            perf_mode=PM.DoubleRow,
                        )
                for x in range(nhalf):
                    nc.tensor.matmul(
                        pss[x],
                        lhsT=ohst_sb[:, :, m * P : (m + 1) * P],
                        rhs=rhs_t[h * nhalf + x][:, kt : kt + 2, :],
                        start=False,
                        stop=True,
                        perf_mode=PM.DoubleRow,
                    )
                for x in range(nhalf):
                    n = h * nhalf + x
                    scr = scrpool.tile([P, nt], f32, tag="scr", name="scr")
                    nc.scalar.copy(out=scr, in_=pss[x])
                    nc.vector.tensor_reduce(
                        out=u_t[:, m, n : n + 1], in_=scr, axis=AT.X,
                        op=OP.min,
                    )
                    nc.vector.tensor_reduce(
                        out=v_t[:, m, n : n + 1], in_=scr, axis=AT.X,
                        op=OP.max,
                    )

        _emit_epilogue_v5(
            nc, small, singles, psum1, u_t, v_t, sql_sb, sqlC_sb, vld_sb,
            onesc, outd, mc,
        )


def _emit_body_v5(
    nc, tc, embT8, ohstd, sqlCd, sqld, vldd, outd,
    b, r, mc, kt, ntil, nt, version=5,
):
    """v5: fp8 e4m3 DoubleRow matmuls.  The moving stream is 18 slabs of 128
    rows: 16 emb k-slabs + one sq-levels slab (rows 0..2 hold a 3-level fp8
    decomposition of -sq_n/2 with weights 16, 1, 1/16) + one +128*one-hot
    (label) slab.  Eight DoubleRow pairs cover the Gram; a ninth pair
    (sq-levels slab, one-hot slab) adds both -sq_n/2 and
    -(C/2)[label_m == label_n] in PSUM, so

        p[m,n] = <Qe_m, Qe_n> - sq_n/2 - (C/2)[same]

    and DVE does just two pure reduces per block: u = rowmin(p),
    v = rowmax(p).  Then dp2 = -2u + sq_m - C, dn2 = -2v + sq_m (score
    max/min swap under the -2 scale).  Column rotation keeps the own-block
    load doubling as the stationary operand."""
    from contextlib import ExitStack

    import concourse.mybir as mybir

    f32 = mybir.dt.float32
    fp8 = mybir.dt.float8e4
    AT = mybir.AxisListType
    OP = mybir.AluOpType
    PM = mybir.MatmulPerfMode
    assert r == nt, "column rotation requires r == nt"
    kt8 = kt + 2  # moving slabs incl. sq-levels + one-hot
    npair = kt // 2

    with ExitStack() as ctx:
        singles = ctx.enter_context(tc.tile_pool(name="singles", bufs=1))
        psum = ctx.enter_context(tc.tile_pool(name="psum", bufs=6, space="PSUM"))
        psum1 = ctx.enter_context(
            tc.tile_pool(name="psum1", bufs=1, space="PSUM")
        )
        small = ctx.enter_context(tc.tile_pool(name="small", bufs=2))
        rhspool = ctx.enter_context(tc.tile_pool(name="rhspool", bufs=3))
        scrpool = ctx.enter_context(tc.tile_pool(name="scrpool", bufs=4))

        # own block: stationary operand AND the n=0 moving operand
        lhs_sb = singles.tile([P, kt8, nt], fp8)
        if version == 6:
            embr = embT8.rearrange("(n p) (k c) -> n p k c", p=P, k=kt8)
            nc.sync.dma_start(out=lhs_sb, in_=embr[0])
        else:
            embr = embT8.rearrange("(k p) n -> k p n", p=P)
            for k in range(kt8):
                nc.sync.dma_start(out=lhs_sb[:, k, :], in_=embr[k, :, 0:nt])
        ohst_sb = singles.tile([P, 2, nt], fp8)
        nc.sync.dma_start(
            out=ohst_sb, in_=ohstd.rearrange("p (two m) -> p two m", two=2)
        )
        sql_sb = singles.tile([P, mc], f32)
        nc.sync.dma_start(out=sql_sb, in_=sqld)
        sqlC_sb = singles.tile([P, mc], f32)
        nc.sync.dma_start(out=sqlC_sb, in_=sqlCd)
        vld_sb = singles.tile([P, mc], f32)
        nc.sync.dma_start(out=vld_sb, in_=vldd)
        onesc = singles.tile([P, 1], f32)
        nc.vector.memset(onesc, 1.0)

        u_t = singles.tile([P, mc, ntil], f32)  # rowmin partials
        v_t = singles.tile([P, mc, ntil], f32)  # rowmax partials
        if ABLATE:
            nc.vector.memset(u_t, 0.0)
            nc.vector.memset(v_t, 0.0)

        for n in range(ntil):
            if n == 0 or ABLATE == "nodma":
                rhs = lhs_sb
            else:
                rhs = rhspool.tile([P, kt8, nt], fp8, tag="rhs", name="rhs")
                if version == 6:
                    nc.sync.dma_start(out=rhs, in_=embr[n])
                else:
                    for k in range(kt8):
                        nc.sync.dma_start(
                            out=rhs[:, k, :],
                            in_=embr[k, :, n * nt : (n + 1) * nt],
                        )
            for m in range(mc):
                ps = psum.tile([P, nt], f32, tag="ps", name="ps")
                if ABLATE != "nope":
                    for j in range(npair):
                        nc.tensor.matmul(
                            ps,
                            lhsT=lhs_sb[
                                :, 2 * j : 2 * j + 2, m * P : (m + 1) * P
                            ],
                            rhs=rhs[:, 2 * j : 2 * j + 2, :],
                            start=(j == 0),
                            stop=False,
                            perf_mode=PM.DoubleRow,
                        )
                    # ninth pair: (sq-levels slab, one-hot slab)
                    nc.tensor.matmul(
                        ps,
                        lhsT=ohst_sb[:, :, m * P : (m + 1) * P],
                        rhs=rhs[:, kt : kt + 2, :],
                        start=False,
                        stop=True,
                        perf_mode=PM.DoubleRow,
                    )
                if ABLATE == "nodve":
                    continue
                if version == 6:
                    # ACT copies PSUM->SBUF so the DVE reduces run in 2x
                    # mode (all-SBUF operands) without the PSUM port stall
                    scr = scrpool.tile([P, nt], f32, tag="scr", name="scr")
                    nc.scalar.copy(out=scr, in_=ps)
                    red = scr
                else:
                    red = ps
                nc.vector.tensor_reduce(
                    out=u_t[:, m, n : n + 1], in_=red, axis=AT.X, op=OP.min
                )
                nc.vector.tensor_reduce(
                    out=v_t[:, m, n : n + 1], in_=red, axis=AT.X, op=OP.max
                )

        _emit_epilogue_v5(
            nc, small, singles, psum1, u_t, v_t, sql_sb, sqlC_sb, vld_sb,
            onesc, outd, mc,
        )


def _emit_epilogue_v5(
    nc, small, singles, psum1, u_t, v_t, sql_sb, sqlC_sb, vld_sb, onesc,
    outd, mc,
):
    """Shared v5/v6/v7 epilogue, vectorized across all mc chunks [P, mc]."""
    import concourse.mybir as mybir

    f32 = mybir.dt.float32
    AT = mybir.AxisListType
    OP = mybir.AluOpType

    u_f = small.tile([P, mc], f32, tag="u_f")
    nc.vector.tensor_reduce(out=u_f, in_=u_t, axis=AT.X, op=OP.min)
    v_f = small.tile([P, mc], f32, tag="v_f")
    nc.vector.tensor_reduce(out=v_f, in_=v_t, axis=AT.X, op=OP.max)
    # dp2 = max(-2u + (sq_l - C), 0); dn2 = max(-2v + sq_l, 0)
    dp2r = small.tile([P, mc], f32, tag="dp2r")
    nc.vector.scalar_tensor_tensor(
        out=dp2r, in0=u_f, scalar=-2.0, in1=sqlC_sb,
        op0=OP.mult, op1=OP.add,
    )
    dp2 = small.tile([P, mc], f32, tag="dp2")
    nc.vector.tensor_scalar_max(out=dp2, in0=dp2r, scalar1=0.0)
    dn2r = small.tile([P, mc], f32, tag="dn2r")
    nc.vector.scalar_tensor_tensor(
        out=dn2r, in0=v_f, scalar=-2.0, in1=sql_sb,
        op0=OP.mult, op1=OP.add,
    )
    dn2 = small.tile([P, mc], f32, tag="dn2")
    nc.vector.tensor_scalar_max(out=dn2, in0=dn2r, scalar1=0.0)
    dp = small.tile([P, mc], f32, tag="dp")
    nc.scalar.sqrt(dp, dp2)
    dn = small.tile([P, mc], f32, tag="dn")
    nc.scalar.sqrt(dn, dn2)
    pr = small.tile([P, mc], f32, tag="pr")
    nc.vector.scalar_tensor_tensor(
        out=pr, in0=dp, scalar=MARGIN, in1=dn,
        op0=OP.add, op1=OP.subtract,
    )
    prr = small.tile([P, mc], f32, tag="prr")
    nc.vector.tensor_scalar_max(out=prr, in0=pr, scalar1=0.0)
    stats = singles.tile([P, mc], f32)
    nc.vector.tensor_mul(out=stats, in0=prr, in1=vld_sb)

    outp = psum1.tile([mc, 1], f32)
    nc.tensor.matmul(outp, lhsT=stats, rhs=onesc, start=True, stop=True)
    out_sb = small.tile([mc, 1], f32, tag="out_sb")
    nc.vector.tensor_copy(out=out_sb, in_=outp)
    nc.sync.dma_start(out=outd, in_=out_sb)


def _emit_body_v4(
    nc, tc, embT2, sqfd, labfd, labld, sqlCd, sqld, vldd, outd,
    b, r, mc, kt, ntil, nt, fdt,
):
    """v4: per-core column rotation puts the core's own block at n=0, so the
    block-0 load doubles as the matmul stationary operand (both operands carry
    the host's -2/C scale; one fused DVE op rescales by -C/2 — exact).  The
    sq/label row broadcasts are generated on PE (K=1 ones-matmul, also a PE
    pre-warm) instead of 128x-redundant broadcast DMA."""
    from contextlib import ExitStack

    import concourse.mybir as mybir

    f32 = mybir.dt.float32
    AT = mybir.AxisListType
    OP = mybir.AluOpType
    assert r == nt, "column rotation requires r == nt"

    with ExitStack() as ctx:
        singles = ctx.enter_context(tc.tile_pool(name="singles", bufs=1))
        psum = ctx.enter_context(tc.tile_pool(name="psum", bufs=6, space="PSUM"))
        psum1 = ctx.enter_context(
            tc.tile_pool(name="psum1", bufs=1, space="PSUM")
        )
        small = ctx.enter_context(tc.tile_pool(name="small", bufs=2))

        # own block: stationary operand AND the n=0 moving operand
        lhs_sb = singles.tile([P, kt, nt], fdt)
        embT2r = embT2.rearrange("(k p) n -> k p n", p=P)
        for k in range(kt):
            nc.sync.dma_start(out=lhs_sb[:, k, :], in_=embT2r[k, :, 0:nt])
        sql_sb = singles.tile([P, mc], f32)
        nc.sync.dma_start(out=sql_sb, in_=sqld)
        sqlC_sb = singles.tile([P, mc], f32)
        nc.sync.dma_start(out=sqlC_sb, in_=sqlCd)
        vld_sb = singles.tile([P, mc], f32)
        nc.sync.dma_start(out=vld_sb, in_=vldd)
        labl_sb = singles.tile([P, mc], f32)
        nc.sync.dma_start(out=labl_sb, in_=labld)
        onesc = singles.tile([P, 1], f32)
        nc.vector.memset(onesc, 1.0)

        rhspool = ctx.enter_context(tc.tile_pool(name="rhspool", bufs=3))
        tmppool = ctx.enter_context(tc.tile_pool(name="tmppool", bufs=4))
        bcpool = ctx.enter_context(tc.tile_pool(name="bcpool", bufs=3))

        qmax = singles.tile([P, mc, ntil], f32)
        qmin = singles.tile([P, mc, ntil], f32)

        for n in range(ntil):
            if n == 0:
                rhs = lhs_sb
            else:
                rhs = rhspool.tile([P, kt, nt], fdt, tag="rhs", name="rhs")
                for k in range(kt):
                    nc.sync.dma_start(
                        out=rhs[:, k, :],
                        in_=embT2r[k, :, n * nt : (n + 1) * nt],
                    )
            sqf_bc = bcpool.tile([P, nt], f32, tag="sqf", name="sqf_bc")
            nc.gpsimd.dma_start(
                out=sqf_bc,
                in_=sqfd[:, n * nt : (n + 1) * nt].partition_broadcast(P),
            )
            lab_bc = bcpool.tile([P, nt], f32, tag="lab", name="lab_bc")
            nc.gpsimd.dma_start(
                out=lab_bc,
                in_=labfd[:, n * nt : (n + 1) * nt].partition_broadcast(P),
            )
            for m in range(mc):
                ps = psum.tile([P, nt], f32, tag="ps", name="ps")
                for k in range(kt):
                    nc.tensor.matmul(
                        ps,
                        lhsT=lhs_sb[:, k, m * P : (m + 1) * P],
                        rhs=rhs[:, k, :],
                        start=(k == 0),
                        stop=(k == kt - 1),
                    )
                # tmp2 = [label_n == label_m] + sq_n/C
                tmp2 = tmppool.tile([P, nt], f32, tag="tmp2", name="tmp2")
                nc.vector.scalar_tensor_tensor(
                    out=tmp2,
                    in0=lab_bc,
                    scalar=labl_sb[:, m : m + 1],
                    in1=sqf_bc,
                    op0=OP.is_equal,
                    op1=OP.add,
                )
                # scr = ps*(-C/2) + tmp2  (undo the double -2/C scaling)
                scr = tmppool.tile([P, nt], f32, tag="scr", name="scr")
                nc.vector.scalar_tensor_tensor(
                    out=scr,
                    in0=ps,
                    scalar=-CBIG / 2.0,
                    in1=tmp2,
                    op0=OP.mult,
                    op1=OP.add,
                )
                nc.vector.tensor_reduce(
                    out=qmax[:, m, n : n + 1], in_=scr, axis=AT.X, op=OP.max
                )
                nc.vector.tensor_reduce(
                    out=qmin[:, m, n : n + 1], in_=scr, axis=AT.X, op=OP.min
                )

        # epilogue, vectorized across all mc chunks at once [P, mc]
        qmaxf = small.tile([P, mc], f32, tag="qmaxf")
        nc.vector.tensor_reduce(out=qmaxf, in_=qmax, axis=AT.X, op=OP.max)
        qminf = small.tile([P, mc], f32, tag="qminf")
        nc.vector.tensor_reduce(out=qminf, in_=qmin, axis=AT.X, op=OP.min)
        # dp2 = max(C*qmax + (sq_l - C), 0); dn2 = max(C*qmin + sq_l, 0)
        dp2r = small.tile([P, mc], f32, tag="dp2r")
        nc.vector.scalar_tensor_tensor(
            out=dp2r, in0=qmaxf, scalar=CBIG, in1=sqlC_sb,
            op0=OP.mult, op1=OP.add,
        )
        dp2 = small.tile([P, mc], f32, tag="dp2")
        nc.vector.tensor_scalar_max(out=dp2, in0=dp2r, scalar1=0.0)
        dn2r = small.tile([P, mc], f32, tag="dn2r")
        nc.vector.scalar_tensor_tensor(
            out=dn2r, in0=qminf, scalar=CBIG, in1=sql_sb,
            op0=OP.mult, op1=OP.add,
        )
        dn2 = small.tile([P, mc], f32, tag="dn2")
        nc.vector.tensor_scalar_max(out=dn2, in0=dn2r, scalar1=0.0)
        dp = small.tile([P, mc], f32, tag="dp")
        nc.scalar.sqrt(dp, dp2)
        dn = small.tile([P, mc], f32, tag="dn")
        nc.scalar.sqrt(dn, dn2)
        pr = small.tile([P, mc], f32, tag="pr")
        nc.vector.scalar_tensor_tensor(
            out=pr, in0=dp, scalar=MARGIN, in1=dn,
            op0=OP.add, op1=OP.subtract,
        )
        prr = small.tile([P, mc], f32, tag="prr")
        nc.vector.tensor_scalar_max(out=prr, in0=pr, scalar1=0.0)
        stats = singles.tile([P, mc], f32)
        nc.vector.tensor_mul(out=stats, in0=prr, in1=vld_sb)

        outp = psum1.tile([mc, 1], f32)
        nc.tensor.matmul(outp, lhsT=stats, rhs=onesc, start=True, stop=True)
        out_sb = small.tile([mc, 1], f32, tag="out_sb")
        nc.vector.tensor_copy(out=out_sb, in_=outp)
        nc.sync.dma_start(out=outd, in_=out_sb)


def _emit_body_v3(
    nc, tc, embT2, lhsTd, sqfd, labfd, labld, sqlCd, sqld, vldd, outd,
    b, r, mc, kt, ntil, nt, fdt,
):
    """v3: PE does only the 16-tile Gram; C*same mask (is_equal on broadcast
    labels) and +sq_n both happen on DVE before the fused max reduce."""
    from contextlib import ExitStack

    import concourse.mybir as mybir

    f32 = mybir.dt.float32
    AT = mybir.AxisListType
    OP = mybir.AluOpType

    with ExitStack() as ctx:
        singles = ctx.enter_context(tc.tile_pool(name="singles", bufs=1))
        rhspool = ctx.enter_context(tc.tile_pool(name="rhspool", bufs=3))
        bcpool = ctx.enter_context(tc.tile_pool(name="bcpool", bufs=3))
        tmppool = ctx.enter_context(tc.tile_pool(name="tmppool", bufs=4))
        psum = ctx.enter_context(tc.tile_pool(name="psum", bufs=6, space="PSUM"))
        psum1 = ctx.enter_context(
            tc.tile_pool(name="psum1", bufs=1, space="PSUM")
        )
        small = ctx.enter_context(tc.tile_pool(name="small", bufs=2))

        lhs_sb = singles.tile([P, kt, r], fdt)
        lhsTr = lhsTd.rearrange("(k p) m -> k p m", p=P)
        for k in range(kt):
            nc.sync.dma_start(out=lhs_sb[:, k, :], in_=lhsTr[k])
        sql_sb = singles.tile([P, mc], f32)
        nc.sync.dma_start(out=sql_sb, in_=sqld)
        sqlC_sb = singles.tile([P, mc], f32)
        nc.sync.dma_start(out=sqlC_sb, in_=sqlCd)
        vld_sb = singles.tile([P, mc], f32)
        nc.sync.dma_start(out=vld_sb, in_=vldd)
        labl_sb = singles.tile([P, mc], f32)
        nc.sync.dma_start(out=labl_sb, in_=labld)
        onesc = singles.tile([P, 1], f32)
        nc.vector.memset(onesc, 1.0)

        qmax = singles.tile([P, mc, ntil], f32)
        qmin = singles.tile([P, mc, ntil], f32)

        embT2r = embT2.rearrange("(k p) n -> k p n", p=P)
        for n in range(ntil):
            rhs = rhspool.tile([P, kt, nt], fdt, tag="rhs")
            for k in range(kt):
                nc.sync.dma_start(
                    out=rhs[:, k, :], in_=embT2r[k, :, n * nt : (n + 1) * nt]
                )
            sqf_bc = bcpool.tile([P, nt], f32, tag="sqf")
            nc.gpsimd.dma_start(
                out=sqf_bc,
                in_=sqfd[:, n * nt : (n + 1) * nt].partition_broadcast(P),
            )
            lab_bc = bcpool.tile([P, nt], f32, tag="lab")
            nc.gpsimd.dma_start(
                out=lab_bc,
                in_=labfd[:, n * nt : (n + 1) * nt].partition_broadcast(P),
            )
            for m in range(mc):
                ps = psum.tile([P, nt], f32, tag="ps")
                for k in range(kt):
                    nc.tensor.matmul(
                        ps,
                        lhsT=lhs_sb[:, k, m * P : (m + 1) * P],
                        rhs=rhs[:, k, :],
                        start=(k == 0),
                        stop=(k == kt - 1),
                    )
                # Scores are scaled by 1/C (host pre-scales rhs by -2/C):
                # tmp2 = [label_n == label_m] + sq_n/C
                tmp2 = tmppool.tile([P, nt], f32, tag="tmp2")
                nc.vector.scalar_tensor_tensor(
                    out=tmp2,
                    in0=lab_bc,
                    scalar=labl_sb[:, m : m + 1],
                    in1=sqf_bc,
                    op0=OP.is_equal,
                    op1=OP.add,
                )
                scr = tmppool.tile([P, nt], f32, tag="scr")
                nc.vector.tensor_add(out=scr, in0=ps, in1=tmp2)
                nc.vector.tensor_reduce(
                    out=qmax[:, m, n : n + 1], in_=scr, axis=AT.X, op=OP.max
                )
                nc.vector.tensor_reduce(
                    out=qmin[:, m, n : n + 1], in_=scr, axis=AT.X, op=OP.min
                )

        stats = singles.tile([P, mc], f32)
        for m in range(mc):
            qmaxf = small.tile([P, 1], f32, tag="qmaxf")
            nc.vector.tensor_reduce(
                out=qmaxf, in_=qmax[:, m, :], axis=AT.X, op=OP.max
            )
            qminf = small.tile([P, 1], f32, tag="qminf")
            nc.vector.tensor_reduce(
                out=qminf, in_=qmin[:, m, :], axis=AT.X, op=OP.min
            )
            # un-scale: dp2 = max(C*qmax + (sq_l - C), 0), dn2 likewise
            dp2r = small.tile([P, 1], f32, tag="dp2r")
            nc.vector.tensor_scalar(
                out=dp2r, in0=qmaxf, scalar1=CBIG,
                scalar2=sqlC_sb[:, m : m + 1], op0=OP.mult, op1=OP.add,
            )
            dp2 = small.tile([P, 1], f32, tag="dp2")
            nc.vector.tensor_scalar_max(out=dp2, in0=dp2r, scalar1=0.0)
            dn2r = small.tile([P, 1], f32, tag="dn2r")
            nc.vector.tensor_scalar(
                out=dn2r, in0=qminf, scalar1=CBIG,
                scalar2=sql_sb[:, m : m + 1], op0=OP.mult, op1=OP.add,
            )
            dn2 = small.tile([P, 1], f32, tag="dn2")
            nc.vector.tensor_scalar_max(out=dn2, in0=dn2r, scalar1=0.0)
            dp = small.tile([P, 1], f32, tag="dp")
            nc.scalar.sqrt(dp, dp2)
            dn = small.tile([P, 1], f32, tag="dn")
            nc.scalar.sqrt(dn, dn2)
            pr = small.tile([P, 1], f32, tag="pr")
            nc.vector.scalar_tensor_tensor(
                out=pr, in0=dp, scalar=MARGIN, in1=dn,
                op0=OP.add, op1=OP.subtract,
            )
            nc.vector.tensor_scalar(
                out=stats[:, m : m + 1], in0=pr, scalar1=0.0,
                scalar2=vld_sb[:, m : m + 1], op0=OP.max, op1=OP.mult,
            )

        outp = psum1.tile([mc, 1], f32)
        nc.tensor.matmul(outp, lhsT=stats, rhs=onesc, start=True, stop=True)
        out_sb = small.tile([mc, 1], f32, tag="out_sb")
        nc.vector.tensor_copy(out=out_sb, in_=outp)
        nc.sync.dma_start(out=outd, in_=out_sb)


def _emit_body_v2(
    nc, tc, embT2, lhsTd, sqfd, sqlCd, sqld, vldd, outd,
    b, r, mc, kt, ntil, nt, fdt,
):
    """v2: onehot rides the feature stream (kt tiles incl. onehot); sq_n is
    added on DVE via tensor_tensor_reduce fused with the row-max."""
    from contextlib import ExitStack

    import concourse.mybir as mybir

    f32 = mybir.dt.float32
    AT = mybir.AxisListType
    OP = mybir.AluOpType

    with ExitStack() as ctx:
        singles = ctx.enter_context(tc.tile_pool(name="singles", bufs=1))
        rhspool = ctx.enter_context(tc.tile_pool(name="rhspool", bufs=3))
        sqfpool = ctx.enter_context(tc.tile_pool(name="sqfpool", bufs=3))
        psum = ctx.enter_context(tc.tile_pool(name="psum", bufs=6, space="PSUM"))
        psum1 = ctx.enter_context(
            tc.tile_pool(name="psum1", bufs=1, space="PSUM")
        )
        small = ctx.enter_context(tc.tile_pool(name="small", bufs=2))

        lhs_sb = singles.tile([P, kt, r], fdt)
        lhsTr = lhsTd.rearrange("(k p) m -> k p m", p=P)
        for k in range(kt):
            nc.sync.dma_start(out=lhs_sb[:, k, :], in_=lhsTr[k])
        sql_sb = singles.tile([P, mc], f32)
        nc.sync.dma_start(out=sql_sb, in_=sqld)
        sqlC_sb = singles.tile([P, mc], f32)
        nc.sync.dma_start(out=sqlC_sb, in_=sqlCd)
        vld_sb = singles.tile([P, mc], f32)
        nc.sync.dma_start(out=vld_sb, in_=vldd)
        onesc = singles.tile([P, 1], f32)
        nc.vector.memset(onesc, 1.0)

        qmax = singles.tile([P, mc, ntil], f32)
        qmin = singles.tile([P, mc, ntil], f32)

        embT2r = embT2.rearrange("(k p) n -> k p n", p=P)
        for n in range(ntil):
            rhs = rhspool.tile([P, kt, nt], fdt, tag="rhs")
            for k in range(kt):
                nc.sync.dma_start(
                    out=rhs[:, k, :], in_=embT2r[k, :, n * nt : (n + 1) * nt]
                )
            sqf_bc = sqfpool.tile([P, nt], f32, tag="sqf")
            nc.gpsimd.dma_start(
                out=sqf_bc,
                in_=sqfd[:, n * nt : (n + 1) * nt].partition_broadcast(P),
            )
            for m in range(mc):
                ps = psum.tile([P, nt], f32, tag="ps")
                for k in range(kt):
                    nc.tensor.matmul(
                        ps,
                        lhsT=lhs_sb[:, k, m * P : (m + 1) * P],
                        rhs=rhs[:, k, :],
                        start=(k == 0),
                        stop=(k == kt - 1),
                    )
                # scr = ps + sq_n (broadcast), then row max/min
                scr = sqfpool.tile([P, nt], f32, tag="scr")
                nc.vector.tensor_add(out=scr, in0=ps, in1=sqf_bc)
                nc.vector.tensor_reduce(
                    out=qmax[:, m, n : n + 1], in_=scr, axis=AT.X, op=OP.max
                )
                nc.vector.tensor_reduce(
                    out=qmin[:, m, n : n + 1], in_=scr, axis=AT.X, op=OP.min
                )

        stats = singles.tile([P, mc], f32)
        for m in range(mc):
            qmaxf = small.tile([P, 1], f32, tag="qmaxf")
            nc.vector.tensor_reduce(
                out=qmaxf, in_=qmax[:, m, :], axis=AT.X, op=OP.max
            )
            qminf = small.tile([P, 1], f32, tag="qminf")
            nc.vector.tensor_reduce(
                out=qminf, in_=qmin[:, m, :], axis=AT.X, op=OP.min
            )
            dp2 = small.tile([P, 1], f32, tag="dp2")
            nc.vector.tensor_scalar(
                out=dp2, in0=qmaxf, scalar1=sqlC_sb[:, m : m + 1],
                scalar2=0.0, op0=OP.add, op1=OP.max,
            )
            dn2 = small.tile([P, 1], f32, tag="dn2")
            nc.vector.tensor_scalar(
                out=dn2, in0=qminf, scalar1=sql_sb[:, m : m + 1],
                scalar2=0.0, op0=OP.add, op1=OP.max,
            )
            dp = small.tile([P, 1], f32, tag="dp")
            nc.scalar.sqrt(dp, dp2)
            dn = small.tile([P, 1], f32, tag="dn")
            nc.scalar.sqrt(dn, dn2)
            pr = small.tile([P, 1], f32, tag="pr")
            nc.vector.scalar_tensor_tensor(
                out=pr, in0=dp, scalar=MARGIN, in1=dn,
                op0=OP.add, op1=OP.subtract,
            )
            nc.vector.tensor_scalar(
                out=stats[:, m : m + 1], in0=pr, scalar1=0.0,
                scalar2=vld_sb[:, m : m + 1], op0=OP.max, op1=OP.mult,
            )

        outp = psum1.tile([mc, 1], f32)
        nc.tensor.matmul(outp, lhsT=stats, rhs=onesc, start=True, stop=True)
        out_sb = small.tile([mc, 1], f32, tag="out_sb")
        nc.vector.tensor_copy(out=out_sb, in_=outp)
        nc.sync.dma_start(out=outd, in_=out_sb)


def _emit_body(
    nc, tc, embT2, lhsTd, ohTd, ohTCd, sqrd, sqlCd, sqld, vldd, outd,
    b, r, mc, kt, ntil, nt, l, fdt,
):
    from contextlib import ExitStack

    import concourse.mybir as mybir

    f32r = fdt
    f32 = mybir.dt.float32
    bf16 = mybir.dt.bfloat16
    AT = mybir.AxisListType
    OP = mybir.AluOpType

    if True:
        with ExitStack() as ctx:
            singles = ctx.enter_context(tc.tile_pool(name="singles", bufs=1))
            rhspool = ctx.enter_context(tc.tile_pool(name="rhspool", bufs=3))
            psum = ctx.enter_context(
                tc.tile_pool(name="psum", bufs=6, space="PSUM")
            )
            psum1 = ctx.enter_context(
                tc.tile_pool(name="psum1", bufs=1, space="PSUM")
            )
            small = ctx.enter_context(tc.tile_pool(name="small", bufs=2))

            # Resident operands
            lhs_sb = singles.tile([P, kt, r], f32r)
            lhsTr = lhsTd.rearrange("(k p) m -> k p m", p=P)
            for k in range(kt):
                nc.sync.dma_start(out=lhs_sb[:, k, :], in_=lhsTr[k])
            oh_sb = singles.tile([l, b], bf16)
            nc.sync.dma_start(out=oh_sb, in_=ohTd)
            ohc_sb = singles.tile([l, r], bf16)
            nc.sync.dma_start(out=ohc_sb, in_=ohTCd)
            sq_sb = singles.tile([2, b], bf16)
            nc.sync.dma_start(out=sq_sb, in_=sqrd)
            sql_sb = singles.tile([P, mc], f32)
            nc.sync.dma_start(out=sql_sb, in_=sqld)
            sqlC_sb = singles.tile([P, mc], f32)
            nc.sync.dma_start(out=sqlC_sb, in_=sqlCd)
            vld_sb = singles.tile([P, mc], f32)
            nc.sync.dma_start(out=vld_sb, in_=vldd)
            ones2 = singles.tile([2, P], bf16)
            nc.vector.memset(ones2, 1.0)
            onesc = singles.tile([P, 1], f32)
            nc.vector.memset(onesc, 1.0)

            # Row max / min partials per (m-chunk, n-tile)
            qmax = singles.tile([P, mc, ntil], f32)
            qmin = singles.tile([P, mc, ntil], f32)

            embT2r = embT2.rearrange("(k p) n -> k p n", p=P)
            for n in range(ntil):
                rhs = rhspool.tile([P, kt, nt], f32r, tag="rhs")
                for k in range(kt):
                    nc.sync.dma_start(
                        out=rhs[:, k, :], in_=embT2r[k, :, n * nt : (n + 1) * nt]
                    )
                for m in range(mc):
                    ps = psum.tile([P, nt], f32, tag="ps")
                    for k in range(kt):
                        nc.tensor.matmul(
                            ps,
                            lhsT=lhs_sb[:, k, m * P : (m + 1) * P],
                            rhs=rhs[:, k, :],
                            start=(k == 0),
                            stop=False,
                        )
                    nc.tensor.matmul(
                        ps,
                        lhsT=ohc_sb[:, m * P : (m + 1) * P],
                        rhs=oh_sb[:, n * nt : (n + 1) * nt],
                        start=False,
                        stop=False,
                    )
                    nc.tensor.matmul(
                        ps,
                        lhsT=ones2,
                        rhs=sq_sb[:, n * nt : (n + 1) * nt],
                        start=False,
                        stop=True,
                    )
                    nc.vector.tensor_reduce(
                        out=qmax[:, m, n : n + 1], in_=ps, axis=AT.X, op=OP.max
                    )
                    nc.vector.tensor_reduce(
                        out=qmin[:, m, n : n + 1], in_=ps, axis=AT.X, op=OP.min
                    )

            # Per-anchor loss tail
            stats = singles.tile([P, mc], f32)
            for m in range(mc):
                qmaxf = small.tile([P, 1], f32, tag="qmaxf")
                nc.vector.tensor_reduce(
                    out=qmaxf, in_=qmax[:, m, :], axis=AT.X, op=OP.max
                )
                qminf = small.tile([P, 1], f32, tag="qminf")
                nc.vector.tensor_reduce(
                    out=qminf, in_=qmin[:, m, :], axis=AT.X, op=OP.min
                )
                # dp2 = max(qmax + (sq_m - C), 0);  dn2 = max(qmin + sq_m, 0)
                dp2 = small.tile([P, 1], f32, tag="dp2")
                nc.vector.tensor_scalar(
                    out=dp2,
                    in0=qmaxf,
                    scalar1=sqlC_sb[:, m : m + 1],
                    scalar2=0.0,
                    op0=OP.add,
                    op1=OP.max,
                )
                dn2 = small.tile([P, 1], f32, tag="dn2")
                nc.vector.tensor_scalar(
                    out=dn2,
                    in0=qminf,
                    scalar1=sql_sb[:, m : m + 1],
                    scalar2=0.0,
                    op0=OP.add,
                    op1=OP.max,
                )
                dp = small.tile([P, 1], f32, tag="dp")
                nc.scalar.sqrt(dp, dp2)
                dn = small.tile([P, 1], f32, tag="dn")
                nc.scalar.sqrt(dn, dn2)
                # per = max((dp + MARGIN) - dn, 0) * valid
                pr = small.tile([P, 1], f32, tag="pr")
                nc.vector.scalar_tensor_tensor(
                    out=pr,
                    in0=dp,
                    scalar=MARGIN,
                    in1=dn,
                    op0=OP.add,
                    op1=OP.subtract,
                )
                nc.vector.tensor_scalar(
                    out=stats[:, m : m + 1],
                    in0=pr,
                    scalar1=0.0,
                    scalar2=vld_sb[:, m : m + 1],
                    op0=OP.max,
                    op1=OP.mult,
                )

            # Partition-sum each m-chunk's masked losses: out[mc,1] = stats.T @ 1
            outp = psum1.tile([mc, 1], f32)
            nc.tensor.matmul(outp, lhsT=stats, rhs=onesc, start=True, stop=True)
            out_sb = small.tile([mc, 1], f32, tag="out_sb")
            nc.vector.tensor_copy(out=out_sb, in_=outp)
            nc.sync.dma_start(out=outd, in_=out_sb)


def _get_nc(b, d, n_cores):
    key = (b, d, n_cores, FEAT_DT, VERSION)
    if key not in _cache:
        _cache[key] = _build(b, d, n_cores)
    return _cache[key]


def _prep_inputs(emb, lab, n_cores):
    """Host-side sharding/layout prep. Returns (in_maps, valid_count)."""
    b, d = emb.shape
    r = b // n_cores
    mc = r // P
    bf16 = ml_dtypes.bfloat16

    fdt_np = np.float32 if FEAT_DT == "f32r" else bf16
    embT = np.ascontiguousarray(emb.T)                       # [d, b] f32
    oh = (np.arange(L)[:, None] == lab[None, :])             # [L, b] bool

    sq64 = (emb.astype(np.float64) ** 2).sum(axis=1)         # [b]
    sq32 = sq64.astype(np.float32)

    counts = np.bincount(lab, minlength=L)
    valid = ((counts[lab] >= 2) & (counts[lab] <= b - 1)).astype(np.float32)

    if VERSION in (5, 6, 7):
        fp8 = ml_dtypes.float8_e4m3
        oh = (np.arange(L)[:, None] == lab[None, :])         # [L, b] bool
        # sq-levels slab: rows 0..2 hold -sq/2 = 16*X1 + X2 + X3/16 in fp8
        vt = (-0.5 * sq32).astype(np.float32)
        X1 = (vt / 16).astype(fp8)
        rr = vt - 16 * X1.astype(np.float32)
        X2 = rr.astype(fp8)
        rr = rr - X2.astype(np.float32)
        X3 = (16 * rr).astype(fp8)
        sqslab = np.zeros((P, b), fp8)
        sqslab[0], sqslab[1], sqslab[2] = X1, X2, X3
        stream = np.concatenate(
            [embT.astype(fp8), sqslab,
             (oh.astype(np.float32) * 128.0).astype(fp8)], axis=0
        )                                                    # [d+2L, b] fp8
    elif VERSION in (3, 4):
        # scores scaled by 1/C on device (exact power-of-2 scaling)
        embT2 = np.ascontiguousarray((-2.0 / CBIG) * embT).astype(fdt_np)
        sqf = (sq32 / np.float32(CBIG))[None, :]             # [1, b]
        labf = lab.astype(np.float32)[None, :]               # [1, b]
    elif VERSION == 2:
        embT2 = np.concatenate(
            [-2.0 * embT, oh.astype(np.float32)], axis=0
        ).astype(fdt_np)                                     # [d+L, b]
        lhsT_full = np.concatenate(
            [embT, oh.astype(np.float32) * CBIG], axis=0
        ).astype(fdt_np)                                     # [d+L, b]
        sqf = sq32[None, :]                                  # [1, b]
    else:
        embT2 = np.ascontiguousarray(-2.0 * embT).astype(fdt_np)
        ohT = oh.astype(bf16)
        ohTC = (oh.astype(np.float32) * CBIG).astype(bf16)
        sq_hi = sq64.astype(bf16)
        sq_lo = (sq64 - sq_hi.astype(np.float64)).astype(bf16)
        sqr = np.ascontiguousarray(np.stack([sq_hi, sq_lo]))  # [2, b] bf16

    in_maps = []
    for i in range(n_cores):
        s = slice(i * r, (i + 1) * r)
        sql = sq32[s].reshape(mc, P).T                        # [P, mc]
        vld = valid[s].reshape(mc, P).T
        m = {
            "sqlCd": np.ascontiguousarray(sql - np.float32(CBIG)),
            "sqld": np.ascontiguousarray(sql),
            "vldd": np.ascontiguousarray(vld),
        }
        if VERSION not in (5, 6, 7):
            m["embT2"] = embT2
        if VERSION in (5, 6, 7):
            fp8 = ml_dtypes.float8_e4m3
            # rotate columns so this core's own block lands at n-tile 0
            rot = np.roll(stream, -i * r, axis=1)
            if VERSION >= 6:
                # pre-tile: [kt8*P, b] -> [ntil*P, kt8*NT] so each n-tile is
                # one contiguous DMA (row n*P+p holds all slabs' NT cols)
                kt8 = stream.shape[0] // P
                m["embT8"] = np.ascontiguousarray(
                    rot.reshape(kt8, P, b // NT, NT)
                    .transpose(2, 1, 0, 3)
                    .reshape(b // NT * P, kt8 * NT)
                )
            else:
                m["embT8"] = np.ascontiguousarray(rot)
            ohst = np.zeros((P, 2, r), np.float32)
            ohst[0, 0, :] = 16.0     # weights for the sq-levels rows
            ohst[1, 0, :] = 1.0
            ohst[2, 0, :] = 1.0 / 16.0
            ohst[:, 1, :] = oh[:, s].astype(np.float32) * -128.0
            m["ohstd"] = np.ascontiguousarray(
                ohst.reshape(P, 2 * r).astype(fp8)
            )
        elif VERSION == 4:
            # rotate columns so this core's own block lands at n-tile 0
            m["embT2"] = np.ascontiguousarray(np.roll(embT2, -i * r, axis=1))
            m["sqfd"] = np.ascontiguousarray(np.roll(sqf, -i * r, axis=1))
            m["labfd"] = np.ascontiguousarray(np.roll(labf, -i * r, axis=1))
            m["labld"] = np.ascontiguousarray(
                lab[s].astype(np.float32).reshape(mc, P).T
            )
        elif VERSION == 3:
            m["lhsTd"] = np.ascontiguousarray(embT[:, s]).astype(fdt_np)
            m["sqfd"] = sqf
            m["labfd"] = labf
            m["labld"] = np.ascontiguousarray(
                lab[s].astype(np.float32).reshape(mc, P).T
            )
        elif VERSION == 2:
            m["lhsTd"] = np.ascontiguousarray(lhsT_full[:, s])
            m["sqfd"] = sqf
        else:
            m["lhsTd"] = np.ascontiguousarray(embT[:, s]).astype(fdt_np)
            m["ohTd"] = ohT
            m["ohTCd"] = np.ascontiguousarray(ohTC[:, s])
            m["sqrd"] = sqr
        in_maps.append(m)
    return in_maps, float(valid.sum())


def kernel(embeddings, labels):
    global LAST_RESULT
    from concourse.bass_utils import run_bass_kernel_spmd

    emb = np.asarray(embeddings, dtype=np.float32)
    lab = np.asarray(labels).astype(np.int64)
    b, d = emb.shape
    n_cores = NCORES

    nc = _get_nc(b, d, n_cores)
    in_maps, cnt = _prep_inputs(emb, lab, n_cores)

    res = run_bass_kernel_spmd(
        nc, in_maps, core_ids=list(range(n_cores)), trace=TRACE
    )
    LAST_RESULT = res

    total = np.float32(0.0)
    for core_out in res.results:
        total += core_out["out"].astype(np.float32).sum()
    if cnt > 0:
        loss = np.float32(total / np.float32(cnt))
    else:
        loss = np.float32(0.0)
    return np.asarray(loss, dtype=np.float32)



# revision 34
# speedup vs baseline: 1.0011x; 1.0011x over previous
"""Batch-hard triplet loss on 8 Trainium2 NeuronCores (Bass/Tile).

Strategy (data-parallel over anchor rows):
  Each core owns R = B/8 anchor rows and mines hard pos/neg from the score
  block  S[m, n] = ||e_m - e_n||^2 + C * [label_m == label_n]  without ever
  materializing indices or gathers:

      hard-positive d2 = rowmax(S) - C - sq_m   (same-label entries at d2+C)
      hard-negative d2 = rowmin(S)     - sq_m   (diff-label entries at d2)

  With C (32768) larger than any squared distance, the diagonal sits at
  exactly ~C: never the max when a real positive exists, never the min when
  a real negative exists; degenerate rows are masked by the host-computed
  `valid`.  Row max/min are tie-immune (values, not argmax indices).

  Default pipeline (VERSION=4):
  - PE streams the pure Gram:  q = ((-2/C)emb.T)^T @ ((-2/C)emb.T) in
    float32r (full PE rate at N=512, near-fp32 precision; the 1/C and the
    later -C/2 rescale are exact power-of-2 scalings).
  - Per-core column ROTATION (host-side roll) puts the core's own 512-col
    block at n-tile 0, so that one load doubles as the matmul stationary
    operand — no separate lhsT input or transfer.
  - The label-equality mask and +sq_n/C ride a single fused DVE op
    ((lab_bc == lab_m) + sq_bc), added to the PSUM block with the -C/2
    rescale in a second fused op, then DVE row max/min reduce.
  - lab/sq rows reach all 128 partitions via stride-0 broadcast DMA
    (GpSimd SWDGE).
  - Loss tail (sqrt via ACT, margin/relu/valid-mask via DVE, partition-sum
    via a ones-matmul) stays on device; the host only sums 4 partial sums
    per core and divides by the valid count.

  Older fallbacks kept for reference: VERSION=1 (bf16 one-hot + hi/lo sq
  k-tiles appended to the matmul), 2 (one-hot folded into the f32r stream),
  3 (v4 without rotation).  FEAT_DT=bf16 halves DMA bytes at ~bf16 matmul
  precision; default stays f32r.
"""

import numpy as np
import ml_dtypes

B = 4096
D = 2048
NCORES = 8
L = 128          # number of label classes (labels are in [0, 128))
P = 128          # partitions
NT = 512         # n-tile (matmul free dim = one PSUM bank of f32)
CBIG = 32768.0   # separation constant; must exceed max squared distance
MARGIN = 0.3

import os as _os

TRACE = False           # test.py sets this to profile
LAST_RESULT = None      # BassKernelResults of the most recent run
# "f32r" (near-fp32 matmul) or "bf16" (half the DMA)
FEAT_DT = _os.environ.get("KERNEL_FEAT_DT", "f32r")
# 1: separate bf16 onehot/sq matmul k-tiles (18 total)
# 2: onehot folded into the feature stream (17 tiles), sq added on DVE
# 3: pure 16-tile Gram on PE; same-mask (is_equal) + sq both on DVE
# 4: v3 + column rotation (own block doubles as lhsT) + on-PE broadcasts
# 5: fp8e4m3 DoubleRow matmuls (2 k-slabs/instr, 2x PE rate); +-128 one-hot
#    mask pair on PE; DVE = 2 fused (ps - sq_n/2) -> min/max passes
# 6: v5 + pre-tiled contiguous DMA (1 descriptor/tile) + ACT PSUM->SBUF copy
#    so DVE reduces run from SBUF in 2x mode
# 7: v6 + weight-stationary loop order (m, j outer; n inner over 4 PSUM
#    banks) so each DoubleRow weight load serves 4 moving streams
#    [NaNs on HW and no speed gain -- walrus does not elide reloads]
# 8: v6 + DoubleRowSwInterleave: stationary weights pre-interleaved
#    (A/B pairs per column, reversed) so LD_WEIGHTS loads both planes in
#    one pass; separate lhsw buffer instead of the rotation-shared tile
VERSION = int(_os.environ.get("KERNEL_VERSION", "8"))
# engine-isolation for bench ablation: "", "nope", "nodma", "nodve"
ABLATE = _os.environ.get("KERNEL_ABLATE", "")

_cache = {}


def _build(b, d, n_cores, l=L, nt=NT, repeat=1, feat=None, version=None):
    """Build + compile the per-core Bass kernel (same NEFF for all cores).

    repeat>1 emits the whole body N times (bench builds: slope timing)."""
    import concourse.mybir as mybir
    import concourse.tile as tile
    from concourse import bacc

    r = b // n_cores      # local anchor rows per core
    mc = r // P           # m-chunks of 128 anchors
    kt = d // P           # feature k-tiles
    ntil = b // nt        # n-tiles over all B columns

    if feat is None:
        feat = FEAT_DT
    if version is None:
        version = VERSION
    f32 = mybir.dt.float32
    bf16 = mybir.dt.bfloat16
    fdt = mybir.dt.float32r if feat == "f32r" else bf16

    nc = bacc.Bacc(
        "TRN2", target_bir_lowering=False, debug=False, num_devices=n_cores
    )

    if version in (5, 6, 7, 8):
        fp8 = mybir.dt.float8e4
        kt8 = d // P + 2  # 16 emb slabs + sq-levels slab + one-hot slab
        if version >= 6:
            # pre-tiled: row (n*P + p), col (k*nt + c)
            embT2 = nc.dram_tensor(
                "embT8", [(b // nt) * P, kt8 * nt], fp8, kind="ExternalInput"
            ).ap()
        else:
            embT2 = nc.dram_tensor(
                "embT8", [kt8 * P, b], fp8, kind="ExternalInput"
            ).ap()
        ohstd = nc.dram_tensor(
            "ohstd", [P, 2 * (b // n_cores)], fp8, kind="ExternalInput"
        ).ap()
        if version == 8:
            lhswd = nc.dram_tensor(
                "lhswd", [P, (d // P // 2 + 1) * (b // n_cores) * 2], fp8,
                kind="ExternalInput",
            ).ap()
    elif version == 4:
        embT2 = nc.dram_tensor("embT2", [d, b], fdt, kind="ExternalInput").ap()
        sqfd = nc.dram_tensor("sqfd", [1, b], f32, kind="ExternalInput").ap()
        labfd = nc.dram_tensor("labfd", [1, b], f32, kind="ExternalInput").ap()
        labld = nc.dram_tensor("labld", [P, b // n_cores // P], f32,
                               kind="ExternalInput").ap()
    elif version == 3:
        embT2 = nc.dram_tensor("embT2", [d, b], fdt, kind="ExternalInput").ap()
        lhsTd = nc.dram_tensor("lhsTd", [d, r], fdt, kind="ExternalInput").ap()
        sqfd = nc.dram_tensor("sqfd", [1, b], f32, kind="ExternalInput").ap()
        labfd = nc.dram_tensor("labfd", [1, b], f32, kind="ExternalInput").ap()
        labld = nc.dram_tensor("labld", [P, b // n_cores // P], f32,
                               kind="ExternalInput").ap()
    elif version == 2:
        d2 = d + l
        embT2 = nc.dram_tensor("embT2", [d2, b], fdt, kind="ExternalInput").ap()
        lhsTd = nc.dram_tensor("lhsTd", [d2, r], fdt, kind="ExternalInput").ap()
        sqfd = nc.dram_tensor("sqfd", [1, b], f32, kind="ExternalInput").ap()
    else:
        embT2 = nc.dram_tensor("embT2", [d, b], fdt, kind="ExternalInput").ap()
        lhsTd = nc.dram_tensor("lhsTd", [d, r], fdt, kind="ExternalInput").ap()
        ohTd = nc.dram_tensor("ohTd", [l, b], bf16, kind="ExternalInput").ap()
        ohTCd = nc.dram_tensor(
            "ohTCd", [l, r], bf16, kind="ExternalInput"
        ).ap()
        sqrd = nc.dram_tensor("sqrd", [2, b], bf16, kind="ExternalInput").ap()
    sqlCd = nc.dram_tensor("sqlCd", [P, mc], f32, kind="ExternalInput").ap()
    sqld = nc.dram_tensor("sqld", [P, mc], f32, kind="ExternalInput").ap()
    vldd = nc.dram_tensor("vldd", [P, mc], f32, kind="ExternalInput").ap()
    outd = nc.dram_tensor("out", [mc, 1], f32, kind="ExternalOutput").ap()

    with tile.TileContext(nc) as tc:
        for _rep in range(repeat):
            if version == 8:
                _emit_body_v8(
                    nc, tc, embT2, lhswd, sqlCd, sqld, vldd, outd,
                    b, r, mc, d // P, ntil, nt,
                )
            elif version == 7:
                _emit_body_v7(
                    nc, tc, embT2, ohstd, sqlCd, sqld, vldd, outd,
                    b, r, mc, d // P, ntil, nt,
                )
            elif version in (5, 6):
                _emit_body_v5(
                    nc, tc, embT2, ohstd, sqlCd, sqld, vldd, outd,
                    b, r, mc, d // P, ntil, nt, version,
                )
            elif version == 4:
                _emit_body_v4(
                    nc, tc, embT2, sqfd, labfd, labld, sqlCd, sqld,
                    vldd, outd, b, r, mc, kt, ntil, nt, fdt,
                )
            elif version == 3:
                _emit_body_v3(
                    nc, tc, embT2, lhsTd, sqfd, labfd, labld, sqlCd, sqld,
                    vldd, outd, b, r, mc, kt, ntil, nt, fdt,
                )
            elif version == 2:
                _emit_body_v2(
                    nc, tc, embT2, lhsTd, sqfd, sqlCd, sqld, vldd, outd,
                    b, r, mc, (d + l) // P, ntil, nt, fdt,
                )
            else:
                _emit_body(
                    nc, tc, embT2, lhsTd, ohTd, ohTCd, sqrd, sqlCd, sqld,
                    vldd, outd, b, r, mc, kt, ntil, nt, l, fdt,
                )

    nc.compile()
    return nc


def _emit_body_v8(
    nc, tc, embT8, lhswd, sqlCd, sqld, vldd, outd,
    b, r, mc, kt, ntil, nt,
):
    """v8: like v6 but stationary weights come from a separate buffer in
    DoubleRowSwInterleave layout (A/B planes interleaved per column, columns
    reversed), so LD_WEIGHTS pulls both DoubleRow planes in one pass."""
    from contextlib import ExitStack

    import concourse.mybir as mybir

    f32 = mybir.dt.float32
    fp8 = mybir.dt.float8e4
    AT = mybir.AxisListType
    OP = mybir.AluOpType
    PM = mybir.MatmulPerfMode
    assert r == nt, "column rotation requires r == nt"
    kt8 = kt + 2
    npair = kt // 2  # gram pairs; pair index npair is the mask/sq pair

    with ExitStack() as ctx:
        singles = ctx.enter_context(tc.tile_pool(name="singles", bufs=1))
        psum = ctx.enter_context(tc.tile_pool(name="psum", bufs=6, space="PSUM"))
        psum1 = ctx.enter_context(
            tc.tile_pool(name="psum1", bufs=1, space="PSUM")
        )
        small = ctx.enter_context(tc.tile_pool(name="small", bufs=2))
        rhspool = ctx.enter_context(tc.tile_pool(name="rhspool", bufs=3))
        scrpool = ctx.enter_context(tc.tile_pool(name="scrpool", bufs=4))

        embr = embT8.rearrange("(n p) (k c) -> n p k c", p=P, k=kt8)
        lhsw_sb = singles.tile([P, npair + 1, mc, 2 * P], fp8)
        nc.sync.dma_start(
            out=lhsw_sb,
            in_=lhswd.rearrange(
                "p (j c t) -> p j c t", j=npair + 1, c=mc
            ),
        )
        lhs_sb = singles.tile([P, kt8, nt], fp8)
        nc.sync.dma_start(out=lhs_sb, in_=embr[0])
        sql_sb = singles.tile([P, mc], f32)
        nc.sync.dma_start(out=sql_sb, in_=sqld)
        sqlC_sb = singles.tile([P, mc], f32)
        nc.sync.dma_start(out=sqlC_sb, in_=sqlCd)
        vld_sb = singles.tile([P, mc], f32)
        nc.sync.dma_start(out=vld_sb, in_=vldd)
        onesc = singles.tile([P, 1], f32)
        nc.vector.memset(onesc, 1.0)

        u_t = singles.tile([P, mc, ntil], f32)  # rowmin partials
        v_t = singles.tile([P, mc, ntil], f32)  # rowmax partials

        for n in range(ntil):
            if n == 0:
                rhs = lhs_sb
            else:
                rhs = rhspool.tile([P, kt8, nt], fp8, tag="rhs", name="rhs")
                nc.sync.dma_start(out=rhs, in_=embr[n])
            for m in range(mc):
                ps = psum.tile([P, nt], f32, tag="ps", name="ps")
                for j in range(npair):
                    nc.tensor.matmul(
                        ps,
                        lhsT=lhsw_sb[:, j, m, :],
                        rhs=rhs[:, 2 * j : 2 * j + 2, :],
                        start=(j == 0),
                        stop=False,
                        perf_mode=PM.DoubleRowSwInterleave,
                    )
                # mask/sq pair
                nc.tensor.matmul(
                    ps,
                    lhsT=lhsw_sb[:, npair, m, :],
                    rhs=rhs[:, kt : kt + 2, :],
                    start=False,
                    stop=True,
                    perf_mode=PM.DoubleRowSwInterleave,
                )
                scr = scrpool.tile([P, nt], f32, tag="scr", name="scr")
                nc.scalar.copy(out=scr, in_=ps)
                nc.vector.tensor_reduce(
                    out=u_t[:, m, n : n + 1], in_=scr, axis=AT.X, op=OP.min
                )
                nc.vector.tensor_reduce(
                    out=v_t[:, m, n : n + 1], in_=scr, axis=AT.X, op=OP.max
                )

        _emit_epilogue_v5(
            nc, small, singles, psum1, u_t, v_t, sql_sb, sqlC_sb, vld_sb,
            onesc, outd, mc,
        )


def _emit_body_v7(
    nc, tc, embT8, ohstd, sqlCd, sqld, vldd, outd,
    b, r, mc, kt, ntil, nt,
):
    """v7: weight-stationary ordering.  All 8 rhs n-tiles stay resident in
    SBUF; for each (m-chunk, half-of-4-n-tiles), the j loop loads each
    DoubleRow weight set once and streams it against 4 moving n-tiles into 4
    PSUM banks (interleaved accumulation groups), cutting LD_WEIGHTS traffic
    8x.  Reduce chain and math identical to v6."""
    from contextlib import ExitStack

    import concourse.mybir as mybir

    f32 = mybir.dt.float32
    fp8 = mybir.dt.float8e4
    AT = mybir.AxisListType
    OP = mybir.AluOpType
    PM = mybir.MatmulPerfMode
    assert r == nt, "column rotation requires r == nt"
    kt8 = kt + 2
    npair = kt // 2
    nhalf = 4  # n-tiles per PSUM wave

    with ExitStack() as ctx:
        singles = ctx.enter_context(tc.tile_pool(name="singles", bufs=1))
        psum = ctx.enter_context(tc.tile_pool(name="psum", bufs=7, space="PSUM"))
        psum1 = ctx.enter_context(
            tc.tile_pool(name="psum1", bufs=1, space="PSUM")
        )
        small = ctx.enter_context(tc.tile_pool(name="small", bufs=2))
        rhspool = ctx.enter_context(tc.tile_pool(name="rhspool", bufs=9))
        scrpool = ctx.enter_context(tc.tile_pool(name="scrpool", bufs=4))

        embr = embT8.rearrange("(n p) (k c) -> n p k c", p=P, k=kt8)
        rhs_t = []
        for n in range(ntil):
            t = rhspool.tile([P, kt8, nt], fp8, tag="rhs", name=f"rhs{n}")
            nc.sync.dma_start(out=t, in_=embr[n])
            rhs_t.append(t)
        lhs_sb = rhs_t[0]  # rotation: own block doubles as stationary
        ohst_sb = singles.tile([P, 2, nt], fp8)
        nc.sync.dma_start(
            out=ohst_sb, in_=ohstd.rearrange("p (two m) -> p two m", two=2)
        )
        sql_sb = singles.tile([P, mc], f32)
        nc.sync.dma_start(out=sql_sb, in_=sqld)
        sqlC_sb = singles.tile([P, mc], f32)
        nc.sync.dma_start(out=sqlC_sb, in_=sqlCd)
        vld_sb = singles.tile([P, mc], f32)
        nc.sync.dma_start(out=vld_sb, in_=vldd)
        onesc = singles.tile([P, 1], f32)
        nc.vector.memset(onesc, 1.0)

        u_t = singles.tile([P, mc, ntil], f32)  # rowmin partials
        v_t = singles.tile([P, mc, ntil], f32)  # rowmax partials

        for m in range(mc):
            for h in range(ntil // nhalf):
                pss = []
                for x in range(nhalf):
                    pss.append(psum.tile([P, nt], f32, tag="ps", name="ps"))
                for j in range(npair):
                    lhsT = lhs_sb[:, 2 * j : 2 * j + 2, m * P : (m + 1) * P]
                    for x in range(nhalf):
                        nc.tensor.matmul(
                            pss[x],
                            lhsT=lhsT,
                            rhs=rhs_t[h * nhalf + x][:, 2 * j : 2 * j + 2, :],
                            start=(j == 0),
                            stop=False,
                            perf_mode=PM.DoubleRow,
                        )
                for x in range(nhalf):
                    nc.tensor.matmul(
                        pss[x],
                        lhsT=ohst_sb[:, :, m * P : (m + 1) * P],
                        rhs=rhs_t[h * nhalf + x][:, kt : kt + 2, :],
                        start=False,
                        stop=True,
                        perf_mode=PM.DoubleRow,
                    )
                for x in range(nhalf):
                    n = h * nhalf + x
                    scr = scrpool.tile([P, nt], f32, tag="scr", name="scr")
                    nc.scalar.copy(out=scr, in_=pss[x])
                    nc.vector.tensor_reduce(
                        out=u_t[:, m, n : n + 1], in_=scr, axis=AT.X,
                        op=OP.min,
                    )
                    nc.vector.tensor_reduce(
                        out=v_t[:, m, n : n + 1], in_=scr, axis=AT.X,
                        op=OP.max,
                    )

        _emit_epilogue_v5(
            nc, small, singles, psum1, u_t, v_t, sql_sb, sqlC_sb, vld_sb,
            onesc, outd, mc,
        )


def _emit_body_v5(
    nc, tc, embT8, ohstd, sqlCd, sqld, vldd, outd,
    b, r, mc, kt, ntil, nt, version=5,
):
    """v5: fp8 e4m3 DoubleRow matmuls.  The moving stream is 18 slabs of 128
    rows: 16 emb k-slabs + one sq-levels slab (rows 0..2 hold a 3-level fp8
    decomposition of -sq_n/2 with weights 16, 1, 1/16) + one +128*one-hot
    (label) slab.  Eight DoubleRow pairs cover the Gram; a ninth pair
    (sq-levels slab, one-hot slab) adds both -sq_n/2 and
    -(C/2)[label_m == label_n] in PSUM, so

        p[m,n] = <Qe_m, Qe_n> - sq_n/2 - (C/2)[same]

    and DVE does just two pure reduces per block: u = rowmin(p),
    v = rowmax(p).  Then dp2 = -2u + sq_m - C, dn2 = -2v + sq_m (score
    max/min swap under the -2 scale).  Column rotation keeps the own-block
    load doubling as the stationary operand."""
    from contextlib import ExitStack

    import concourse.mybir as mybir

    f32 = mybir.dt.float32
    fp8 = mybir.dt.float8e4
    AT = mybir.AxisListType
    OP = mybir.AluOpType
    PM = mybir.MatmulPerfMode
    assert r == nt, "column rotation requires r == nt"
    kt8 = kt + 2  # moving slabs incl. sq-levels + one-hot
    npair = kt // 2

    with ExitStack() as ctx:
        singles = ctx.enter_context(tc.tile_pool(name="singles", bufs=1))
        psum = ctx.enter_context(tc.tile_pool(name="psum", bufs=6, space="PSUM"))
        psum1 = ctx.enter_context(
            tc.tile_pool(name="psum1", bufs=1, space="PSUM")
        )
        small = ctx.enter_context(tc.tile_pool(name="small", bufs=2))
        rhspool = ctx.enter_context(tc.tile_pool(name="rhspool", bufs=3))
        scrpool = ctx.enter_context(tc.tile_pool(name="scrpool", bufs=4))

        # own block: stationary operand AND the n=0 moving operand
        lhs_sb = singles.tile([P, kt8, nt], fp8)
        if version == 6:
            embr = embT8.rearrange("(n p) (k c) -> n p k c", p=P, k=kt8)
            nc.sync.dma_start(out=lhs_sb, in_=embr[0])
        else:
            embr = embT8.rearrange("(k p) n -> k p n", p=P)
            for k in range(kt8):
                nc.sync.dma_start(out=lhs_sb[:, k, :], in_=embr[k, :, 0:nt])
        ohst_sb = singles.tile([P, 2, nt], fp8)
        nc.sync.dma_start(
            out=ohst_sb, in_=ohstd.rearrange("p (two m) -> p two m", two=2)
        )
        sql_sb = singles.tile([P, mc], f32)
        nc.sync.dma_start(out=sql_sb, in_=sqld)
        sqlC_sb = singles.tile([P, mc], f32)
        nc.sync.dma_start(out=sqlC_sb, in_=sqlCd)
        vld_sb = singles.tile([P, mc], f32)
        nc.sync.dma_start(out=vld_sb, in_=vldd)
        onesc = singles.tile([P, 1], f32)
        nc.vector.memset(onesc, 1.0)

        u_t = singles.tile([P, mc, ntil], f32)  # rowmin partials
        v_t = singles.tile([P, mc, ntil], f32)  # rowmax partials
        if ABLATE:
            nc.vector.memset(u_t, 0.0)
            nc.vector.memset(v_t, 0.0)

        for n in range(ntil):
            if n == 0 or ABLATE == "nodma":
                rhs = lhs_sb
            else:
                rhs = rhspool.tile([P, kt8, nt], fp8, tag="rhs", name="rhs")
                if version == 6:
                    nc.sync.dma_start(out=rhs, in_=embr[n])
                else:
                    for k in range(kt8):
                        nc.sync.dma_start(
                            out=rhs[:, k, :],
                            in_=embr[k, :, n * nt : (n + 1) * nt],
                        )
            for m in range(mc):
                ps = psum.tile([P, nt], f32, tag="ps", name="ps")
                if ABLATE != "nope":
                    for j in range(npair):
                        nc.tensor.matmul(
                            ps,
                            lhsT=lhs_sb[
                                :, 2 * j : 2 * j + 2, m * P : (m + 1) * P
                            ],
                            rhs=rhs[:, 2 * j : 2 * j + 2, :],
                            start=(j == 0),
                            stop=False,
                            perf_mode=PM.DoubleRow,
                        )
                    # ninth pair: (sq-levels slab, one-hot slab)
                    nc.tensor.matmul(
                        ps,
                        lhsT=ohst_sb[:, :, m * P : (m + 1) * P],
                        rhs=rhs[:, kt : kt + 2, :],
                        start=False,
                        stop=True,
                        perf_mode=PM.DoubleRow,
                    )
                if ABLATE == "nodve":
                    continue
                if version == 6:
                    # ACT copies PSUM->SBUF so the DVE reduces run in 2x
                    # mode (all-SBUF operands) without the PSUM port stall
                    scr = scrpool.tile([P, nt], f32, tag="scr", name="scr")
                    nc.scalar.copy(out=scr, in_=ps)
                    red = scr
                else:
                    red = ps
                nc.vector.tensor_reduce(
                    out=u_t[:, m, n : n + 1], in_=red, axis=AT.X, op=OP.min
                )
                nc.vector.tensor_reduce(
                    out=v_t[:, m, n : n + 1], in_=red, axis=AT.X, op=OP.max
                )

        _emit_epilogue_v5(
            nc, small, singles, psum1, u_t, v_t, sql_sb, sqlC_sb, vld_sb,
            onesc, outd, mc,
        )


def _emit_epilogue_v5(
    nc, small, singles, psum1, u_t, v_t, sql_sb, sqlC_sb, vld_sb, onesc,
    outd, mc,
):
    """Shared v5/v6/v7 epilogue, vectorized across all mc chunks [P, mc]."""
    import concourse.mybir as mybir

    f32 = mybir.dt.float32
    AT = mybir.AxisListType
    OP = mybir.AluOpType

    u_f = small.tile([P, mc], f32, tag="u_f")
    nc.vector.tensor_reduce(out=u_f, in_=u_t, axis=AT.X, op=OP.min)
    v_f = small.tile([P, mc], f32, tag="v_f")
    nc.vector.tensor_reduce(out=v_f, in_=v_t, axis=AT.X, op=OP.max)
    # dp2 = max(-2u + (sq_l - C), 0); dn2 = max(-2v + sq_l, 0)
    dp2r = small.tile([P, mc], f32, tag="dp2r")
    nc.vector.scalar_tensor_tensor(
        out=dp2r, in0=u_f, scalar=-2.0, in1=sqlC_sb,
        op0=OP.mult, op1=OP.add,
    )
    dp2 = small.tile([P, mc], f32, tag="dp2")
    nc.vector.tensor_scalar_max(out=dp2, in0=dp2r, scalar1=0.0)
    dn2r = small.tile([P, mc], f32, tag="dn2r")
    nc.vector.scalar_tensor_tensor(
        out=dn2r, in0=v_f, scalar=-2.0, in1=sql_sb,
        op0=OP.mult, op1=OP.add,
    )
    dn2 = small.tile([P, mc], f32, tag="dn2")
    nc.vector.tensor_scalar_max(out=dn2, in0=dn2r, scalar1=0.0)
    dp = small.tile([P, mc], f32, tag="dp")
    nc.scalar.sqrt(dp, dp2)
    dn = small.tile([P, mc], f32, tag="dn")
    nc.scalar.sqrt(dn, dn2)
    pr = small.tile([P, mc], f32, tag="pr")
    nc.vector.scalar_tensor_tensor(
        out=pr, in0=dp, scalar=MARGIN, in1=dn,
        op0=OP.add, op1=OP.subtract,
    )
    prr = small.tile([P, mc], f32, tag="prr")
    nc.vector.tensor_scalar_max(out=prr, in0=pr, scalar1=0.0)
    stats = singles.tile([P, mc], f32)
    nc.vector.tensor_mul(out=stats, in0=prr, in1=vld_sb)

    outp = psum1.tile([mc, 1], f32)
    nc.tensor.matmul(outp, lhsT=stats, rhs=onesc, start=True, stop=True)
    out_sb = small.tile([mc, 1], f32, tag="out_sb")
    nc.vector.tensor_copy(out=out_sb, in_=outp)
    nc.sync.dma_start(out=outd, in_=out_sb)


def _emit_body_v4(
    nc, tc, embT2, sqfd, labfd, labld, sqlCd, sqld, vldd, outd,
    b, r, mc, kt, ntil, nt, fdt,
):
    """v4: per-core column rotation puts the core's own block at n=0, so the
    block-0 load doubles as the matmul stationary operand (both operands carry
    the host's -2/C scale; one fused DVE op rescales by -C/2 — exact).  The
    sq/label row broadcasts are generated on PE (K=1 ones-matmul, also a PE
    pre-warm) instead of 128x-redundant broadcast DMA."""
    from contextlib import ExitStack

    import concourse.mybir as mybir

    f32 = mybir.dt.float32
    AT = mybir.AxisListType
    OP = mybir.AluOpType
    assert r == nt, "column rotation requires r == nt"

    with ExitStack() as ctx:
        singles = ctx.enter_context(tc.tile_pool(name="singles", bufs=1))
        psum = ctx.enter_context(tc.tile_pool(name="psum", bufs=6, space="PSUM"))
        psum1 = ctx.enter_context(
            tc.tile_pool(name="psum1", bufs=1, space="PSUM")
        )
        small = ctx.enter_context(tc.tile_pool(name="small", bufs=2))

        # own block: stationary operand AND the n=0 moving operand
        lhs_sb = singles.tile([P, kt, nt], fdt)
        embT2r = embT2.rearrange("(k p) n -> k p n", p=P)
        for k in range(kt):
            nc.sync.dma_start(out=lhs_sb[:, k, :], in_=embT2r[k, :, 0:nt])
        sql_sb = singles.tile([P, mc], f32)
        nc.sync.dma_start(out=sql_sb, in_=sqld)
        sqlC_sb = singles.tile([P, mc], f32)
        nc.sync.dma_start(out=sqlC_sb, in_=sqlCd)
        vld_sb = singles.tile([P, mc], f32)
        nc.sync.dma_start(out=vld_sb, in_=vldd)
        labl_sb = singles.tile([P, mc], f32)
        nc.sync.dma_start(out=labl_sb, in_=labld)
        onesc = singles.tile([P, 1], f32)
        nc.vector.memset(onesc, 1.0)

        rhspool = ctx.enter_context(tc.tile_pool(name="rhspool", bufs=3))
        tmppool = ctx.enter_context(tc.tile_pool(name="tmppool", bufs=4))
        bcpool = ctx.enter_context(tc.tile_pool(name="bcpool", bufs=3))

        qmax = singles.tile([P, mc, ntil], f32)
        qmin = singles.tile([P, mc, ntil], f32)

        for n in range(ntil):
            if n == 0:
                rhs = lhs_sb
            else:
                rhs = rhspool.tile([P, kt, nt], fdt, tag="rhs", name="rhs")
                for k in range(kt):
                    nc.sync.dma_start(
                        out=rhs[:, k, :],
                        in_=embT2r[k, :, n * nt : (n + 1) * nt],
                    )
            sqf_bc = bcpool.tile([P, nt], f32, tag="sqf", name="sqf_bc")
            nc.gpsimd.dma_start(
                out=sqf_bc,
                in_=sqfd[:, n * nt : (n + 1) * nt].partition_broadcast(P),
            )
            lab_bc = bcpool.tile([P, nt], f32, tag="lab", name="lab_bc")
            nc.gpsimd.dma_start(
                out=lab_bc,
                in_=labfd[:, n * nt : (n + 1) * nt].partition_broadcast(P),
            )
            for m in range(mc):
                ps = psum.tile([P, nt], f32, tag="ps", name="ps")
                for k in range(kt):
                    nc.tensor.matmul(
                        ps,
                        lhsT=lhs_sb[:, k, m * P : (m + 1) * P],
                        rhs=rhs[:, k, :],
                        start=(k == 0),
                        stop=(k == kt - 1),
                    )
                # tmp2 = [label_n == label_m] + sq_n/C
                tmp2 = tmppool.tile([P, nt], f32, tag="tmp2", name="tmp2")
                nc.vector.scalar_tensor_tensor(
                    out=tmp2,
                    in0=lab_bc,
                    scalar=labl_sb[:, m : m + 1],
                    in1=sqf_bc,
                    op0=OP.is_equal,
                    op1=OP.add,
                )
                # scr = ps*(-C/2) + tmp2  (undo the double -2/C scaling)
                scr = tmppool.tile([P, nt], f32, tag="scr", name="scr")
                nc.vector.scalar_tensor_tensor(
                    out=scr,
                    in0=ps,
                    scalar=-CBIG / 2.0,
                    in1=tmp2,
                    op0=OP.mult,
                    op1=OP.add,
                )
                nc.vector.tensor_reduce(
                    out=qmax[:, m, n : n + 1], in_=scr, axis=AT.X, op=OP.max
                )
                nc.vector.tensor_reduce(
                    out=qmin[:, m, n : n + 1], in_=scr, axis=AT.X, op=OP.min
                )

        # epilogue, vectorized across all mc chunks at once [P, mc]
        qmaxf = small.tile([P, mc], f32, tag="qmaxf")
        nc.vector.tensor_reduce(out=qmaxf, in_=qmax, axis=AT.X, op=OP.max)
        qminf = small.tile([P, mc], f32, tag="qminf")
        nc.vector.tensor_reduce(out=qminf, in_=qmin, axis=AT.X, op=OP.min)
        # dp2 = max(C*qmax + (sq_l - C), 0); dn2 = max(C*qmin + sq_l, 0)
        dp2r = small.tile([P, mc], f32, tag="dp2r")
        nc.vector.scalar_tensor_tensor(
            out=dp2r, in0=qmaxf, scalar=CBIG, in1=sqlC_sb,
            op0=OP.mult, op1=OP.add,
        )
        dp2 = small.tile([P, mc], f32, tag="dp2")
        nc.vector.tensor_scalar_max(out=dp2, in0=dp2r, scalar1=0.0)
        dn2r = small.tile([P, mc], f32, tag="dn2r")
        nc.vector.scalar_tensor_tensor(
            out=dn2r, in0=qminf, scalar=CBIG, in1=sql_sb,
            op0=OP.mult, op1=OP.add,
        )
        dn2 = small.tile([P, mc], f32, tag="dn2")
        nc.vector.tensor_scalar_max(out=dn2, in0=dn2r, scalar1=0.0)
        dp = small.tile([P, mc], f32, tag="dp")
        nc.scalar.sqrt(dp, dp2)
        dn = small.tile([P, mc], f32, tag="dn")
        nc.scalar.sqrt(dn, dn2)
        pr = small.tile([P, mc], f32, tag="pr")
        nc.vector.scalar_tensor_tensor(
            out=pr, in0=dp, scalar=MARGIN, in1=dn,
            op0=OP.add, op1=OP.subtract,
        )
        prr = small.tile([P, mc], f32, tag="prr")
        nc.vector.tensor_scalar_max(out=prr, in0=pr, scalar1=0.0)
        stats = singles.tile([P, mc], f32)
        nc.vector.tensor_mul(out=stats, in0=prr, in1=vld_sb)

        outp = psum1.tile([mc, 1], f32)
        nc.tensor.matmul(outp, lhsT=stats, rhs=onesc, start=True, stop=True)
        out_sb = small.tile([mc, 1], f32, tag="out_sb")
        nc.vector.tensor_copy(out=out_sb, in_=outp)
        nc.sync.dma_start(out=outd, in_=out_sb)


def _emit_body_v3(
    nc, tc, embT2, lhsTd, sqfd, labfd, labld, sqlCd, sqld, vldd, outd,
    b, r, mc, kt, ntil, nt, fdt,
):
    """v3: PE does only the 16-tile Gram; C*same mask (is_equal on broadcast
    labels) and +sq_n both happen on DVE before the fused max reduce."""
    from contextlib import ExitStack

    import concourse.mybir as mybir

    f32 = mybir.dt.float32
    AT = mybir.AxisListType
    OP = mybir.AluOpType

    with ExitStack() as ctx:
        singles = ctx.enter_context(tc.tile_pool(name="singles", bufs=1))
        rhspool = ctx.enter_context(tc.tile_pool(name="rhspool", bufs=3))
        bcpool = ctx.enter_context(tc.tile_pool(name="bcpool", bufs=3))
        tmppool = ctx.enter_context(tc.tile_pool(name="tmppool", bufs=4))
        psum = ctx.enter_context(tc.tile_pool(name="psum", bufs=6, space="PSUM"))
        psum1 = ctx.enter_context(
            tc.tile_pool(name="psum1", bufs=1, space="PSUM")
        )
        small = ctx.enter_context(tc.tile_pool(name="small", bufs=2))

        lhs_sb = singles.tile([P, kt, r], fdt)
        lhsTr = lhsTd.rearrange("(k p) m -> k p m", p=P)
        for k in range(kt):
            nc.sync.dma_start(out=lhs_sb[:, k, :], in_=lhsTr[k])
        sql_sb = singles.tile([P, mc], f32)
        nc.sync.dma_start(out=sql_sb, in_=sqld)
        sqlC_sb = singles.tile([P, mc], f32)
        nc.sync.dma_start(out=sqlC_sb, in_=sqlCd)
        vld_sb = singles.tile([P, mc], f32)
        nc.sync.dma_start(out=vld_sb, in_=vldd)
        labl_sb = singles.tile([P, mc], f32)
        nc.sync.dma_start(out=labl_sb, in_=labld)
        onesc = singles.tile([P, 1], f32)
        nc.vector.memset(onesc, 1.0)

        qmax = singles.tile([P, mc, ntil], f32)
        qmin = singles.tile([P, mc, ntil], f32)

        embT2r = embT2.rearrange("(k p) n -> k p n", p=P)
        for n in range(ntil):
            rhs = rhspool.tile([P, kt, nt], fdt, tag="rhs")
            for k in range(kt):
                nc.sync.dma_start(
                    out=rhs[:, k, :], in_=embT2r[k, :, n * nt : (n + 1) * nt]
                )
            sqf_bc = bcpool.tile([P, nt], f32, tag="sqf")
            nc.gpsimd.dma_start(
                out=sqf_bc,
                in_=sqfd[:, n * nt : (n + 1) * nt].partition_broadcast(P),
            )
            lab_bc = bcpool.tile([P, nt], f32, tag="lab")
            nc.gpsimd.dma_start(
                out=lab_bc,
                in_=labfd[:, n * nt : (n + 1) * nt].partition_broadcast(P),
            )
            for m in range(mc):
                ps = psum.tile([P, nt], f32, tag="ps")
                for k in range(kt):
                    nc.tensor.matmul(
                        ps,
                        lhsT=lhs_sb[:, k, m * P : (m + 1) * P],
                        rhs=rhs[:, k, :],
                        start=(k == 0),
                        stop=(k == kt - 1),
                    )
                # Scores are scaled by 1/C (host pre-scales rhs by -2/C):
                # tmp2 = [label_n == label_m] + sq_n/C
                tmp2 = tmppool.tile([P, nt], f32, tag="tmp2")
                nc.vector.scalar_tensor_tensor(
                    out=tmp2,
                    in0=lab_bc,
                    scalar=labl_sb[:, m : m + 1],
                    in1=sqf_bc,
                    op0=OP.is_equal,
                    op1=OP.add,
                )
                scr = tmppool.tile([P, nt], f32, tag="scr")
                nc.vector.tensor_add(out=scr, in0=ps, in1=tmp2)
                nc.vector.tensor_reduce(
                    out=qmax[:, m, n : n + 1], in_=scr, axis=AT.X, op=OP.max
                )
                nc.vector.tensor_reduce(
                    out=qmin[:, m, n : n + 1], in_=scr, axis=AT.X, op=OP.min
                )

        stats = singles.tile([P, mc], f32)
        for m in range(mc):
            qmaxf = small.tile([P, 1], f32, tag="qmaxf")
            nc.vector.tensor_reduce(
                out=qmaxf, in_=qmax[:, m, :], axis=AT.X, op=OP.max
            )
            qminf = small.tile([P, 1], f32, tag="qminf")
            nc.vector.tensor_reduce(
                out=qminf, in_=qmin[:, m, :], axis=AT.X, op=OP.min
            )
            # un-scale: dp2 = max(C*qmax + (sq_l - C), 0), dn2 likewise
            dp2r = small.tile([P, 1], f32, tag="dp2r")
            nc.vector.tensor_scalar(
                out=dp2r, in0=qmaxf, scalar1=CBIG,
                scalar2=sqlC_sb[:, m : m + 1], op0=OP.mult, op1=OP.add,
            )
            dp2 = small.tile([P, 1], f32, tag="dp2")
            nc.vector.tensor_scalar_max(out=dp2, in0=dp2r, scalar1=0.0)
            dn2r = small.tile([P, 1], f32, tag="dn2r")
            nc.vector.tensor_scalar(
                out=dn2r, in0=qminf, scalar1=CBIG,
                scalar2=sql_sb[:, m : m + 1], op0=OP.mult, op1=OP.add,
            )
            dn2 = small.tile([P, 1], f32, tag="dn2")
            nc.vector.tensor_scalar_max(out=dn2, in0=dn2r, scalar1=0.0)
            dp = small.tile([P, 1], f32, tag="dp")
            nc.scalar.sqrt(dp, dp2)
            dn = small.tile([P, 1], f32, tag="dn")
            nc.scalar.sqrt(dn, dn2)
            pr = small.tile([P, 1], f32, tag="pr")
            nc.vector.scalar_tensor_tensor(
                out=pr, in0=dp, scalar=MARGIN, in1=dn,
                op0=OP.add, op1=OP.subtract,
            )
            nc.vector.tensor_scalar(
                out=stats[:, m : m + 1], in0=pr, scalar1=0.0,
                scalar2=vld_sb[:, m : m + 1], op0=OP.max, op1=OP.mult,
            )

        outp = psum1.tile([mc, 1], f32)
        nc.tensor.matmul(outp, lhsT=stats, rhs=onesc, start=True, stop=True)
        out_sb = small.tile([mc, 1], f32, tag="out_sb")
        nc.vector.tensor_copy(out=out_sb, in_=outp)
        nc.sync.dma_start(out=outd, in_=out_sb)


def _emit_body_v2(
    nc, tc, embT2, lhsTd, sqfd, sqlCd, sqld, vldd, outd,
    b, r, mc, kt, ntil, nt, fdt,
):
    """v2: onehot rides the feature stream (kt tiles incl. onehot); sq_n is
    added on DVE via tensor_tensor_reduce fused with the row-max."""
    from contextlib import ExitStack

    import concourse.mybir as mybir

    f32 = mybir.dt.float32
    AT = mybir.AxisListType
    OP = mybir.AluOpType

    with ExitStack() as ctx:
        singles = ctx.enter_context(tc.tile_pool(name="singles", bufs=1))
        rhspool = ctx.enter_context(tc.tile_pool(name="rhspool", bufs=3))
        sqfpool = ctx.enter_context(tc.tile_pool(name="sqfpool", bufs=3))
        psum = ctx.enter_context(tc.tile_pool(name="psum", bufs=6, space="PSUM"))
        psum1 = ctx.enter_context(
            tc.tile_pool(name="psum1", bufs=1, space="PSUM")
        )
        small = ctx.enter_context(tc.tile_pool(name="small", bufs=2))

        lhs_sb = singles.tile([P, kt, r], fdt)
        lhsTr = lhsTd.rearrange("(k p) m -> k p m", p=P)
        for k in range(kt):
            nc.sync.dma_start(out=lhs_sb[:, k, :], in_=lhsTr[k])
        sql_sb = singles.tile([P, mc], f32)
        nc.sync.dma_start(out=sql_sb, in_=sqld)
        sqlC_sb = singles.tile([P, mc], f32)
        nc.sync.dma_start(out=sqlC_sb, in_=sqlCd)
        vld_sb = singles.tile([P, mc], f32)
        nc.sync.dma_start(out=vld_sb, in_=vldd)
        onesc = singles.tile([P, 1], f32)
        nc.vector.memset(onesc, 1.0)

        qmax = singles.tile([P, mc, ntil], f32)
        qmin = singles.tile([P, mc, ntil], f32)

        embT2r = embT2.rearrange("(k p) n -> k p n", p=P)
        for n in range(ntil):
            rhs = rhspool.tile([P, kt, nt], fdt, tag="rhs")
            for k in range(kt):
                nc.sync.dma_start(
                    out=rhs[:, k, :], in_=embT2r[k, :, n * nt : (n + 1) * nt]
                )
            sqf_bc = sqfpool.tile([P, nt], f32, tag="sqf")
            nc.gpsimd.dma_start(
                out=sqf_bc,
                in_=sqfd[:, n * nt : (n + 1) * nt].partition_broadcast(P),
            )
            for m in range(mc):
                ps = psum.tile([P, nt], f32, tag="ps")
                for k in range(kt):
                    nc.tensor.matmul(
                        ps,
                        lhsT=lhs_sb[:, k, m * P : (m + 1) * P],
                        rhs=rhs[:, k, :],
                        start=(k == 0),
                        stop=(k == kt - 1),
                    )
                # scr = ps + sq_n (broadcast), then row max/min
                scr = sqfpool.tile([P, nt], f32, tag="scr")
                nc.vector.tensor_add(out=scr, in0=ps, in1=sqf_bc)
                nc.vector.tensor_reduce(
                    out=qmax[:, m, n : n + 1], in_=scr, axis=AT.X, op=OP.max
                )
                nc.vector.tensor_reduce(
                    out=qmin[:, m, n : n + 1], in_=scr, axis=AT.X, op=OP.min
                )

        stats = singles.tile([P, mc], f32)
        for m in range(mc):
            qmaxf = small.tile([P, 1], f32, tag="qmaxf")
            nc.vector.tensor_reduce(
                out=qmaxf, in_=qmax[:, m, :], axis=AT.X, op=OP.max
            )
            qminf = small.tile([P, 1], f32, tag="qminf")
            nc.vector.tensor_reduce(
                out=qminf, in_=qmin[:, m, :], axis=AT.X, op=OP.min
            )
            dp2 = small.tile([P, 1], f32, tag="dp2")
            nc.vector.tensor_scalar(
                out=dp2, in0=qmaxf, scalar1=sqlC_sb[:, m : m + 1],
                scalar2=0.0, op0=OP.add, op1=OP.max,
            )
            dn2 = small.tile([P, 1], f32, tag="dn2")
            nc.vector.tensor_scalar(
                out=dn2, in0=qminf, scalar1=sql_sb[:, m : m + 1],
                scalar2=0.0, op0=OP.add, op1=OP.max,
            )
            dp = small.tile([P, 1], f32, tag="dp")
            nc.scalar.sqrt(dp, dp2)
            dn = small.tile([P, 1], f32, tag="dn")
            nc.scalar.sqrt(dn, dn2)
            pr = small.tile([P, 1], f32, tag="pr")
            nc.vector.scalar_tensor_tensor(
                out=pr, in0=dp, scalar=MARGIN, in1=dn,
                op0=OP.add, op1=OP.subtract,
            )
            nc.vector.tensor_scalar(
                out=stats[:, m : m + 1], in0=pr, scalar1=0.0,
                scalar2=vld_sb[:, m : m + 1], op0=OP.max, op1=OP.mult,
            )

        outp = psum1.tile([mc, 1], f32)
        nc.tensor.matmul(outp, lhsT=stats, rhs=onesc, start=True, stop=True)
        out_sb = small.tile([mc, 1], f32, tag="out_sb")
        nc.vector.tensor_copy(out=out_sb, in_=outp)
        nc.sync.dma_start(out=outd, in_=out_sb)


def _emit_body(
    nc, tc, embT2, lhsTd, ohTd, ohTCd, sqrd, sqlCd, sqld, vldd, outd,
    b, r, mc, kt, ntil, nt, l, fdt,
):
    from contextlib import ExitStack

    import concourse.mybir as mybir

    f32r = fdt
    f32 = mybir.dt.float32
    bf16 = mybir.dt.bfloat16
    AT = mybir.AxisListType
    OP = mybir.AluOpType

    if True:
        with ExitStack() as ctx:
            singles = ctx.enter_context(tc.tile_pool(name="singles", bufs=1))
            rhspool = ctx.enter_context(tc.tile_pool(name="rhspool", bufs=3))
            psum = ctx.enter_context(
                tc.tile_pool(name="psum", bufs=6, space="PSUM")
            )
            psum1 = ctx.enter_context(
                tc.tile_pool(name="psum1", bufs=1, space="PSUM")
            )
            small = ctx.enter_context(tc.tile_pool(name="small", bufs=2))

            # Resident operands
            lhs_sb = singles.tile([P, kt, r], f32r)
            lhsTr = lhsTd.rearrange("(k p) m -> k p m", p=P)
            for k in range(kt):
                nc.sync.dma_start(out=lhs_sb[:, k, :], in_=lhsTr[k])
            oh_sb = singles.tile([l, b], bf16)
            nc.sync.dma_start(out=oh_sb, in_=ohTd)
            ohc_sb = singles.tile([l, r], bf16)
            nc.sync.dma_start(out=ohc_sb, in_=ohTCd)
            sq_sb = singles.tile([2, b], bf16)
            nc.sync.dma_start(out=sq_sb, in_=sqrd)
            sql_sb = singles.tile([P, mc], f32)
            nc.sync.dma_start(out=sql_sb, in_=sqld)
            sqlC_sb = singles.tile([P, mc], f32)
            nc.sync.dma_start(out=sqlC_sb, in_=sqlCd)
            vld_sb = singles.tile([P, mc], f32)
            nc.sync.dma_start(out=vld_sb, in_=vldd)
            ones2 = singles.tile([2, P], bf16)
            nc.vector.memset(ones2, 1.0)
            onesc = singles.tile([P, 1], f32)
            nc.vector.memset(onesc, 1.0)

            # Row max / min partials per (m-chunk, n-tile)
            qmax = singles.tile([P, mc, ntil], f32)
            qmin = singles.tile([P, mc, ntil], f32)

            embT2r = embT2.rearrange("(k p) n -> k p n", p=P)
            for n in range(ntil):
                rhs = rhspool.tile([P, kt, nt], f32r, tag="rhs")
                for k in range(kt):
                    nc.sync.dma_start(
                        out=rhs[:, k, :], in_=embT2r[k, :, n * nt : (n + 1) * nt]
                    )
                for m in range(mc):
                    ps = psum.tile([P, nt], f32, tag="ps")
                    for k in range(kt):
                        nc.tensor.matmul(
                            ps,
                            lhsT=lhs_sb[:, k, m * P : (m + 1) * P],
                            rhs=rhs[:, k, :],
                            start=(k == 0),
                            stop=False,
                        )
                    nc.tensor.matmul(
                        ps,
                        lhsT=ohc_sb[:, m * P : (m + 1) * P],
                        rhs=oh_sb[:, n * nt : (n + 1) * nt],
                        start=False,
                        stop=False,
                    )
                    nc.tensor.matmul(
                        ps,
                        lhsT=ones2,
                        rhs=sq_sb[:, n * nt : (n + 1) * nt],
                        start=False,
                        stop=True,
                    )
                    nc.vector.tensor_reduce(
                        out=qmax[:, m, n : n + 1], in_=ps, axis=AT.X, op=OP.max
                    )
                    nc.vector.tensor_reduce(
                        out=qmin[:, m, n : n + 1], in_=ps, axis=AT.X, op=OP.min
                    )

            # Per-anchor loss tail
            stats = singles.tile([P, mc], f32)
            for m in range(mc):
                qmaxf = small.tile([P, 1], f32, tag="qmaxf")
                nc.vector.tensor_reduce(
                    out=qmaxf, in_=qmax[:, m, :], axis=AT.X, op=OP.max
                )
                qminf = small.tile([P, 1], f32, tag="qminf")
                nc.vector.tensor_reduce(
                    out=qminf, in_=qmin[:, m, :], axis=AT.X, op=OP.min
                )
                # dp2 = max(qmax + (sq_m - C), 0);  dn2 = max(qmin + sq_m, 0)
                dp2 = small.tile([P, 1], f32, tag="dp2")
                nc.vector.tensor_scalar(
                    out=dp2,
                    in0=qmaxf,
                    scalar1=sqlC_sb[:, m : m + 1],
                    scalar2=0.0,
                    op0=OP.add,
                    op1=OP.max,
                )
                dn2 = small.tile([P, 1], f32, tag="dn2")
                nc.vector.tensor_scalar(
                    out=dn2,
                    in0=qminf,
                    scalar1=sql_sb[:, m : m + 1],
                    scalar2=0.0,
                    op0=OP.add,
                    op1=OP.max,
                )
                dp = small.tile([P, 1], f32, tag="dp")
                nc.scalar.sqrt(dp, dp2)
                dn = small.tile([P, 1], f32, tag="dn")
                nc.scalar.sqrt(dn, dn2)
                # per = max((dp + MARGIN) - dn, 0) * valid
                pr = small.tile([P, 1], f32, tag="pr")
                nc.vector.scalar_tensor_tensor(
                    out=pr,
                    in0=dp,
                    scalar=MARGIN,
                    in1=dn,
                    op0=OP.add,
                    op1=OP.subtract,
                )
                nc.vector.tensor_scalar(
                    out=stats[:, m : m + 1],
                    in0=pr,
                    scalar1=0.0,
                    scalar2=vld_sb[:, m : m + 1],
                    op0=OP.max,
                    op1=OP.mult,
                )

            # Partition-sum each m-chunk's masked losses: out[mc,1] = stats.T @ 1
            outp = psum1.tile([mc, 1], f32)
            nc.tensor.matmul(outp, lhsT=stats, rhs=onesc, start=True, stop=True)
            out_sb = small.tile([mc, 1], f32, tag="out_sb")
            nc.vector.tensor_copy(out=out_sb, in_=outp)
            nc.sync.dma_start(out=outd, in_=out_sb)


def _get_nc(b, d, n_cores):
    key = (b, d, n_cores, FEAT_DT, VERSION)
    if key not in _cache:
        _cache[key] = _build(b, d, n_cores)
    return _cache[key]


def _prep_inputs(emb, lab, n_cores):
    """Host-side sharding/layout prep. Returns (in_maps, valid_count)."""
    b, d = emb.shape
    r = b // n_cores
    mc = r // P
    bf16 = ml_dtypes.bfloat16

    fdt_np = np.float32 if FEAT_DT == "f32r" else bf16
    embT = np.ascontiguousarray(emb.T)                       # [d, b] f32
    oh = (np.arange(L)[:, None] == lab[None, :])             # [L, b] bool

    sq64 = (emb.astype(np.float64) ** 2).sum(axis=1)         # [b]
    sq32 = sq64.astype(np.float32)

    counts = np.bincount(lab, minlength=L)
    valid = ((counts[lab] >= 2) & (counts[lab] <= b - 1)).astype(np.float32)

    if VERSION in (5, 6, 7, 8):
        fp8 = ml_dtypes.float8_e4m3
        oh = (np.arange(L)[:, None] == lab[None, :])         # [L, b] bool
        # sq-levels slab: rows 0..2 hold -sq/2 = 16*X1 + X2 + X3/16 in fp8
        vt = (-0.5 * sq32).astype(np.float32)
        X1 = (vt / 16).astype(fp8)
        rr = vt - 16 * X1.astype(np.float32)
        X2 = rr.astype(fp8)
        rr = rr - X2.astype(np.float32)
        X3 = (16 * rr).astype(fp8)
        sqslab = np.zeros((P, b), fp8)
        sqslab[0], sqslab[1], sqslab[2] = X1, X2, X3
        stream = np.concatenate(
            [embT.astype(fp8), sqslab,
             (oh.astype(np.float32) * 128.0).astype(fp8)], axis=0
        )                                                    # [d+2L, b] fp8
    elif VERSION in (3, 4):
        # scores scaled by 1/C on device (exact power-of-2 scaling)
        embT2 = np.ascontiguousarray((-2.0 / CBIG) * embT).astype(fdt_np)
        sqf = (sq32 / np.float32(CBIG))[None, :]             # [1, b]
        labf = lab.astype(np.float32)[None, :]               # [1, b]
    elif VERSION == 2:
        embT2 = np.concatenate(
            [-2.0 * embT, oh.astype(np.float32)], axis=0
        ).astype(fdt_np)                                     # [d+L, b]
        lhsT_full = np.concatenate(
            [embT, oh.astype(np.float32) * CBIG], axis=0
        ).astype(fdt_np)                                     # [d+L, b]
        sqf = sq32[None, :]                                  # [1, b]
    else:
        embT2 = np.ascontiguousarray(-2.0 * embT).astype(fdt_np)
        ohT = oh.astype(bf16)
        ohTC = (oh.astype(np.float32) * CBIG).astype(bf16)
        sq_hi = sq64.astype(bf16)
        sq_lo = (sq64 - sq_hi.astype(np.float64)).astype(bf16)
        sqr = np.ascontiguousarray(np.stack([sq_hi, sq_lo]))  # [2, b] bf16

    in_maps = []
    for i in range(n_cores):
        s = slice(i * r, (i + 1) * r)
        sql = sq32[s].reshape(mc, P).T                        # [P, mc]
        vld = valid[s].reshape(mc, P).T
        m = {
            "sqlCd": np.ascontiguousarray(sql - np.float32(CBIG)),
            "sqld": np.ascontiguousarray(sql),
            "vldd": np.ascontiguousarray(vld),
        }
        if VERSION not in (5, 6, 7, 8):
            m["embT2"] = embT2
        if VERSION in (5, 6, 7, 8):
            fp8 = ml_dtypes.float8_e4m3
            # rotate columns so this core's own block lands at n-tile 0
            rot = np.roll(stream, -i * r, axis=1)
            if VERSION >= 6:
                # pre-tile: [kt8*P, b] -> [ntil*P, kt8*NT] so each n-tile is
                # one contiguous DMA (row n*P+p holds all slabs' NT cols)
                kt8 = stream.shape[0] // P
                m["embT8"] = np.ascontiguousarray(
                    rot.reshape(kt8, P, b // NT, NT)
                    .transpose(2, 1, 0, 3)
                    .reshape(b // NT * P, kt8 * NT)
                )
            else:
                m["embT8"] = np.ascontiguousarray(rot)
            ohst = np.zeros((P, 2, r), np.float32)
            ohst[0, 0, :] = 16.0     # weights for the sq-levels rows
            ohst[1, 0, :] = 1.0
            ohst[2, 0, :] = 1.0 / 16.0
            ohst[:, 1, :] = oh[:, s].astype(np.float32) * -128.0
            m["ohstd"] = np.ascontiguousarray(
                ohst.reshape(P, 2 * r).astype(fp8)
            )
            if VERSION == 8:
                # SwInterleave stationary: per (pair, m-chunk) window the
                # 256 cols are [A127, B127, A126, B126, ..., A0, B0]
                kt = stream.shape[0] // P - 2
                npair = kt // 2
                mcl = r // P
                planes = np.empty((npair + 1, 2, P, r), np.float32)
                for j in range(npair):
                    planes[j, 0] = rot[2 * j * P : (2 * j + 1) * P, 0:r]
                    planes[j, 1] = rot[(2 * j + 1) * P : (2 * j + 2) * P, 0:r]
                planes[npair, 0] = ohst[:, 0, :]
                planes[npair, 1] = ohst[:, 1, :]
                rev = planes.reshape(npair + 1, 2, P, mcl, P)[..., ::-1]
                lhsw = rev.transpose(2, 0, 3, 4, 1).reshape(
                    P, (npair + 1) * mcl * 2 * P
                )
                m["lhswd"] = np.ascontiguousarray(lhsw.astype(fp8))
        elif VERSION == 4:
            # rotate columns so this core's own block lands at n-tile 0
            m["embT2"] = np.ascontiguousarray(np.roll(embT2, -i * r, axis=1))
            m["sqfd"] = np.ascontiguousarray(np.roll(sqf, -i * r, axis=1))
            m["labfd"] = np.ascontiguousarray(np.roll(labf, -i * r, axis=1))
            m["labld"] = np.ascontiguousarray(
                lab[s].astype(np.float32).reshape(mc, P).T
            )
        elif VERSION == 3:
            m["lhsTd"] = np.ascontiguousarray(embT[:, s]).astype(fdt_np)
            m["sqfd"] = sqf
            m["labfd"] = labf
            m["labld"] = np.ascontiguousarray(
                lab[s].astype(np.float32).reshape(mc, P).T
            )
        elif VERSION == 2:
            m["lhsTd"] = np.ascontiguousarray(lhsT_full[:, s])
            m["sqfd"] = sqf
        else:
            m["lhsTd"] = np.ascontiguousarray(embT[:, s]).astype(fdt_np)
            m["ohTd"] = ohT
            m["ohTCd"] = np.ascontiguousarray(ohTC[:, s])
            m["sqrd"] = sqr
        in_maps.append(m)
    return in_maps, float(valid.sum())


def kernel(embeddings, labels):
    global LAST_RESULT
    from concourse.bass_utils import run_bass_kernel_spmd

    emb = np.asarray(embeddings, dtype=np.float32)
    lab = np.asarray(labels).astype(np.int64)
    b, d = emb.shape
    n_cores = NCORES

    nc = _get_nc(b, d, n_cores)
    in_maps, cnt = _prep_inputs(emb, lab, n_cores)

    res = run_bass_kernel_spmd(
        nc, in_maps, core_ids=list(range(n_cores)), trace=TRACE
    )
    LAST_RESULT = res

    total = np.float32(0.0)
    for core_out in res.results:
        total += core_out["out"].astype(np.float32).sum()
    if cnt > 0:
        loss = np.float32(total / np.float32(cnt))
    else:
        loss = np.float32(0.0)
    return np.asarray(loss, dtype=np.float32)

